# revision 16
# baseline (speedup 1.0000x reference)
"""CronRoot (sqrt-N block-sparse causal) multihead attention on 8 trn2 cores.

v3 (from v2 base): k-projection moves to fp8 DoubleRow like q (measured
rel-err 1.84e-2 vs the 2e-2 gate, verified against a bit-matched numpy
emulation of the kernel numerics); softmax denominator is computed with
ones-block stationaries that broadcast l directly to [128,512] PSUM,
dropping the separate broadcast matmul, the [2,512] reciprocal and the
av PSUM->SBUF staging copy; the three phases share one pool scope so the
tile scheduler overlaps wave-1 attention with the second half of the
projections; DMA instruction count cut ~5x by host-side re-layout of
weights/x so each tensor loads with one large-descriptor DMA.

Sharding: sequence-parallel. Each core owns 8 of the 64 blocks (512
positions) for all batches/heads; summary k/v recomputed per-core from the
256 summary rows of x (no collectives).

Engine split per (b, head-pair) attention instance:
  PE: 8 local-score mm, 1 summary-score mm (block-diag ksd), 3 denominator
      mm (broadcast-l), 9 AV mm (block-diag vsd + 8 local).
  Scalar: exp(local [128,1024]), exp(summary [128,512]).
  GpSimd: local mask multiply. DVE: summary mask multiply, reciprocal
  [128,512], final (av*1/l) -> bf16 attnT.
"""

import numpy as np
import ml_dtypes
from contextlib import ExitStack

import concourse.bass as bass  # noqa: F401
import concourse.tile as tile
from concourse import bacc, mybir
from concourse.bass_utils import run_bass_kernel_spmd

F32 = mybir.dt.float32
FP8 = mybir.dt.float8e4
DR = mybir.MatmulPerfMode.DoubleRow
QSC = 1.0 / 65536.0  # undo x*16 and w*4096 scaling
FP16 = mybir.dt.float16
AF = mybir.ActivationFunctionType

B, S, D = 4, 4096, 1024
H, HD = 16, 64
BLK = 64                 # block size (= sqrt(S))
NB = S // BLK            # 64 blocks
NCORES = 8
SC = S // NCORES         # 512 seq positions per core
BPC = NB // NCORES       # 8 blocks per core
TC = B * SC              # 2048 (b-major) t columns per core
NSUM = B * NB            # 256 summary positions (b-major)
SCALE = 1.0 / np.sqrt(HD)


def build_nc(repeat=1, phases=(1, 2, 3), k_fp8=True):
    nc = bacc.Bacc("TRN2", target_bir_lowering=False, debug=False,
                   num_devices=NCORES)

    xT = nc.dram_tensor("xT", [128, 8, TC], FP16, kind="ExternalInput").ap()
    xT8 = nc.dram_tensor("xT8", [128, 8, TC], FP8, kind="ExternalInput").ap()
    xsT = nc.dram_tensor("xsT", [128, 8, NSUM], FP16,
                         kind="ExternalInput").ap()
    xs8 = nc.dram_tensor("xs8", [128, 8, NSUM], FP8,
                         kind="ExternalInput").ap()
    wq8 = nc.dram_tensor("wq8", [D, 8, 128], FP8, kind="ExternalInput").ap()
    wk8 = nc.dram_tensor("wk8", [D, 8, 128], FP8, kind="ExternalInput").ap()
    wkT = nc.dram_tensor("wkT", [D, 8, 128], FP16, kind="ExternalInput").ap()
    wvT = nc.dram_tensor("wvT", [256, 8, 512], FP16,
                         kind="ExternalInput").ap()
    biT = nc.dram_tensor("biT", [128, 16], F32, kind="ExternalInput").ap()
    woT = nc.dram_tensor("woT", [128, 8, D], FP16, kind="ExternalInput").ap()
    boT = nc.dram_tensor("boT", [128, 8], F32, kind="ExternalInput").ap()
    cstN = nc.dram_tensor("cstN", [128, 192], FP16, kind="ExternalInput").ap()
    mcomb = nc.dram_tensor("mcomb", [128, 1536], FP16,
                           kind="ExternalInput").ap()
    outT = nc.dram_tensor("outT", [D, TC], FP16, kind="ExternalOutput").ap()

    with tile.TileContext(nc) as tc_:
      for _rep in range(repeat):
       with ExitStack() as ctx:
        pp = ctx.enter_context(tc_.tile_pool(name="persist", bufs=1))
        qT = pp.tile([128, 8, TC], FP16, tag="qT")
        kT = pp.tile([128, 8, TC], FP16, tag="kT")
        v_sb = pp.tile([128, 16, 16, HD], FP16, tag="v")     # (tcn, h, d)
        ksd = pp.tile([128, 32, 128], FP16, tag="ksd")       # (hp*4+b) diag
        vsd = pp.tile([128, 32, 128], FP16, tag="vsd")
        attnT = pp.tile([128, 8, TC], FP16, tag="attnT")

        pw = ctx.enter_context(tc_.tile_pool(name="wstage", bufs=2))
        p2 = ctx.enter_context(tc_.tile_pool(name="p2", bufs=2))
        psq = ctx.enter_context(tc_.tile_pool(name="ps", bufs=1,
                                              space="PSUM"))
        px_cm = tc_.tile_pool(name="xstage", bufs=1)
        px = px_cm.__enter__()

        # critical-path loads first: x chunks feed the first q/k matmuls;
        # split across the SP and Activation HWDGE queues to halve latency
        xT8_sb = px.tile([128, 8, TC], FP8, tag="xT8")
        for dc in range(4):
            eng = nc.sync if dc % 2 == 0 else nc.scalar
            eng.dma_start(xT8_sb[:, 2 * dc:2 * dc + 2, :],
                          xT8[:, 2 * dc:2 * dc + 2, :])
        biT_sb = pp.tile([128, 16], F32, tag="biT")
        nc.sync.dma_start(biT_sb[:], biT[:])
        xsT_sb = px.tile([128, 8, NSUM], FP16, tag="xsT")
        nc.scalar.dma_start(xsT_sb[:], xsT[:])
        xs8_sb = px.tile([128, 8, NSUM], FP8, tag="xs8")
        nc.scalar.dma_start(xs8_sb[:], xs8[:])
        boT_sb = pp.tile([128, 8], F32, tag="boT")
        cst_sb = pp.tile([128, 192], FP16, tag="cstN")
        mcomb_sb = pp.tile([128, 1536], FP16, tag="mcomb")
        nc.vector.memset(ksd[:], 0.0)
        nc.vector.memset(vsd[:], 0.0)

        # ---------------- P1 projections for one half ----------------
        def p1_half(vh):
            for hp in range(4 * vh, 4 * vh + 4):
                # q chunk: fp8 DoubleRow (x*16, wq*4096)
                wq_sb = pw.tile([128, 8, 128], FP8, tag="w8_sb")
                nc.sync.dma_start(wq_sb[:],
                                  wq8[hp * 128:(hp + 1) * 128, :, :])
                for tt in range(4):
                    ps_qk = psq.tile([128, 512], F32, tag="p512", bufs=3)
                    for kp in range(4):
                        nc.tensor.matmul(
                            ps_qk[:],
                            wq_sb[:, 2 * kp:2 * kp + 2, :],
                            xT8_sb[:, 2 * kp:2 * kp + 2,
                                   tt * 512:(tt + 1) * 512],
                            start=(kp == 0), stop=(kp == 3),
                            perf_mode=DR)
                    nc.scalar.activation(
                        qT[:, hp, tt * 512:(tt + 1) * 512], ps_qk[:],
                        AF.Identity, scale=QSC,
                        bias=biT_sb[:, hp:hp + 1])
                # k chunk (+ summary keys into block-diag ksd)
                if k_fp8:
                    wk_sb = pw.tile([128, 8, 128], FP8, tag="wk_sb")
                    nc.sync.dma_start(wk_sb[:],
                                      wk8[hp * 128:(hp + 1) * 128, :, :])
                    for tt in range(4):
                        ps_qk = psq.tile([128, 512], F32, tag="p512", bufs=3)
                        for kp in range(4):
                            nc.tensor.matmul(
                                ps_qk[:],
                                wk_sb[:, 2 * kp:2 * kp + 2, :],
                                xT8_sb[:, 2 * kp:2 * kp + 2,
                                       tt * 512:(tt + 1) * 512],
                                start=(kp == 0), stop=(kp == 3),
                                perf_mode=DR)
                        nc.scalar.activation(
                            kT[:, hp, tt * 512:(tt + 1) * 512], ps_qk[:],
                            AF.Identity, scale=QSC,
                            bias=biT_sb[:, hp + 8:hp + 9])
                    ps_ks = psq.tile([128, 4, 64], F32, tag="p512", bufs=3)
                    for kp in range(4):
                        nc.tensor.matmul(
                            ps_ks[:],
                            wk_sb[:, 2 * kp:2 * kp + 2, :],
                            xs8_sb[:, 2 * kp:2 * kp + 2, :],
                            start=(kp == 0), stop=(kp == 3),
                            perf_mode=DR)
                    ksc = QSC
                else:
                    wk_sb = pw.tile([128, 8, 128], FP16, tag="wk_sb")
                    nc.sync.dma_start(wk_sb[:],
                                      wkT[hp * 128:(hp + 1) * 128, :, :])
                    for tt in range(4):
                        kx = pw.tile([128, 8, 512], FP16, tag="kx")
                        nc.sync.dma_start(kx[:],
                                          xT[:, :, tt * 512:(tt + 1) * 512])
                        ps_qk = psq.tile([128, 512], F32, tag="p512", bufs=3)
                        for dc in range(8):
                            nc.tensor.matmul(
                                ps_qk[:], wk_sb[:, dc, :],
                                kx[:, dc, :],
                                start=(dc == 0), stop=(dc == 7))
                        nc.scalar.activation(
                            kT[:, hp, tt * 512:(tt + 1) * 512], ps_qk[:],
                            AF.Identity, bias=biT_sb[:, hp + 8:hp + 9])
                    ps_ks = psq.tile([128, 4, 64], F32, tag="p512", bufs=3)
                    for dc in range(8):
                        nc.tensor.matmul(ps_ks[:], wk_sb[:, dc, :],
                                         xsT_sb[:, dc, :],
                                         start=(dc == 0), stop=(dc == 7))
                    ksc = 1.0
                nc.scalar.activation(
                    ksd[0:64, hp * 4:hp * 4 + 4, 0:64], ps_ks[0:64],
                    AF.Identity, scale=ksc, bias=biT_sb[0:64, hp + 8:hp + 9])
                nc.scalar.activation(
                    ksd[64:128, hp * 4:hp * 4 + 4, 64:128], ps_ks[64:128],
                    AF.Identity, scale=ksc,
                    bias=biT_sb[64:128, hp + 8:hp + 9])

            # v features for this half (heads 8*vh .. 8*vh+8)
            wv_sb = pw.tile([128, 8, 512], FP16, tag="wv_sb", bufs=1)
            nc.sync.dma_start(wv_sb[:], wvT[vh * 128:(vh + 1) * 128, :, :])
            for tcn in range(16):
                xch = pw.tile([128, 8, 128], FP16, tag="xch", bufs=3)
                nc.sync.dma_start(xch[:],
                                  xT[:, :, tcn * 128:(tcn + 1) * 128])
                ps_v = psq.tile([128, 512], F32, tag="p512", bufs=3)
                for dc in range(8):
                    nc.tensor.matmul(
                        ps_v[:],
                        xch[:, dc, :],
                        wv_sb[:, dc, :],
                        start=(dc == 0), stop=(dc == 7))
                nc.scalar.copy(
                    v_sb[:, tcn, vh * 8:(vh + 1) * 8, :], ps_v[:])
            # summary v -> vs2 (with duplicated partition halves)
            vs2 = px.tile([128, 4, 8, HD], FP16, tag="vs2")
            for sch in range(2):
                ps_vs = psq.tile([128, 512], F32, tag="p512", bufs=3)
                for dc in range(8):
                    nc.tensor.matmul(
                        ps_vs[:],
                        xsT_sb[:, dc, sch * 128:(sch + 1) * 128],
                        wv_sb[:, dc, :],
                        start=(dc == 0), stop=(dc == 7))
                nc.scalar.copy(vs2[0:64, 2 * sch, :, :], ps_vs[0:64])
                nc.scalar.copy(vs2[64:128, 2 * sch + 1, :, :],
                               ps_vs[64:128])
                nc.sync.dma_start(vs2[64:128, 2 * sch, :, :],
                                  vs2[0:64, 2 * sch, :, :])
                nc.sync.dma_start(vs2[0:64, 2 * sch + 1, :, :],
                                  vs2[64:128, 2 * sch + 1, :, :])
            # block-diag summary-v stationaries for this vh's head pairs
            for hp in range(4 * vh, 4 * vh + 4):
                hl = 2 * hp - 8 * vh        # head index within vs2 cols
                nc.vector.tensor_copy(
                    vsd[0:64, hp * 4:hp * 4 + 4, 0:64],
                    vs2[0:64, :, hl, :])
                nc.vector.tensor_copy(
                    vsd[64:128, hp * 4:hp * 4 + 4, 64:128],
                    vs2[64:128, :, hl + 1, :])

        # ------------- P2 attention stages -------------
        def stage_scores(b, hp):
            c0 = b * 512
            s_loc = psq.tile([128, 1024], F32, tag="u1024", bufs=1)
            for hh in range(2):
                for p4 in range(4):
                    cq = c0 + p4 * 128
                    nc.tensor.matmul(
                        s_loc[:, hh * 512 + p4 * 128:
                              hh * 512 + (p4 + 1) * 128],
                        kT[hh * 64:hh * 64 + 64, hp, cq:cq + 128],
                        qT[hh * 64:hh * 64 + 64, hp, cq:cq + 128],
                        start=True, stop=True)
            s_sum = psq.tile([128, 512], F32, tag="u512", bufs=3)
            nc.tensor.matmul(s_sum[:], ksd[:, hp * 4 + b, :],
                             qT[:, hp, c0:c0 + 512],
                             start=True, stop=True)
            pml = p2.tile([128, 1024], FP16, tag="pml")
            nc.scalar.activation(pml[:], s_loc[:], AF.Exp, scale=SCALE)
            pms = p2.tile([128, 512], FP16, tag="pms")
            nc.scalar.activation(pms[:], s_sum[:], AF.Exp, scale=SCALE)
            nc.gpsimd.tensor_mul(pml[:], pml[:], mcomb_sb[:, 0:1024])
            nc.vector.tensor_mul(pms[:], pms[:], mcomb_sb[:, 1024:1536])
            return pml, pms

        def stage_av(b, hp, pml, pms):
            c0 = b * 512
            l_bc = psq.tile([128, 512], F32, tag="u512", bufs=3)
            nc.tensor.matmul(l_bc[:], cst_sb[:, 0:128], pms[:],
                             start=True, stop=False, skip_group_check=True)
            nc.tensor.matmul(l_bc[0:64, :], cst_sb[:, 128:192],
                             pml[:, 0:512],
                             start=False, stop=False, skip_group_check=True)
            nc.tensor.matmul(l_bc[64:128, :], cst_sb[:, 128:192],
                             pml[:, 512:1024],
                             start=False, stop=True, skip_group_check=True)
            rinv = p2.tile([128, 512], FP16, tag="rinv")
            with nc.allow_low_precision(reason="fp16 softmax recip"):
                nc.vector.reciprocal(rinv[:], l_bc[:])
            av = psq.tile([128, 512], F32, tag="u512", bufs=3)
            nc.tensor.matmul(av[:], vsd[:, hp * 4 + b, :], pms[:],
                             start=True, stop=False, skip_group_check=True)
            for hh in range(2):
                for p4 in range(4):
                    nc.tensor.matmul(
                        av[hh * 64:hh * 64 + 64, p4 * 128:(p4 + 1) * 128],
                        v_sb[:, b * 4 + p4, 2 * hp + hh, :],
                        pml[:, hh * 512 + p4 * 128:
                            hh * 512 + (p4 + 1) * 128],
                        start=False, stop=(p4 == 3),
                        skip_group_check=True)
            nc.vector.tensor_mul(attnT[:, hp, c0:c0 + 512], av[:], rinv[:])

        def stage_p3(b):
            c0 = b * 512
            for oc in range(8):
                ps_o = psq.tile([128, 512], F32, tag="p512", bufs=3)
                for hp in range(8):
                    nc.tensor.matmul(
                        ps_o[:],
                        woT_sb[:, hp, oc * 128:(oc + 1) * 128],
                        attnT[:, hp, c0:c0 + 512],
                        start=(hp == 0), stop=(hp == 7))
                o_sb = p2.tile([128, 512], FP16, tag="o_sb", bufs=2)
                nc.scalar.activation(o_sb[:], ps_o[:], AF.Identity,
                                     bias=boT_sb[:, oc:oc + 1])
                nc.sync.dma_start(
                    outT[oc * 128:(oc + 1) * 128, c0:c0 + 512], o_sb[:])

        def run_wave(pairs, with_p3):
            pend = stage_scores(*pairs[0])
            for i, (b, hp) in enumerate(pairs):
                if i + 1 < len(pairs):
                    nxt = stage_scores(*pairs[i + 1])
                else:
                    nxt = None
                stage_av(b, hp, *pend)
                pend = nxt
                if with_p3 and hp == 7:
                    stage_p3(b)

        do2 = 2 in phases
        do3 = 3 in phases
        if 1 in phases:
            p1_half(0)
            nc.sync.dma_start(cst_sb[:], cstN[:])
            nc.sync.dma_start(mcomb_sb[:], mcomb[:])
            nc.sync.dma_start(boT_sb[:], boT[:])
            if do2:
                run_wave([(b, hp) for b in range(B) for hp in range(4)],
                         False)
            p1_half(1)
            px_cm.__exit__(None, None, None)
            pl = ctx.enter_context(tc_.tile_pool(name="late", bufs=1))
            woT_sb = pl.tile([128, 8, D], FP16, tag="woT")
            nc.sync.dma_start(woT_sb[:], woT[:])
            if do2:
                run_wave([(b, hp) for b in range(B) for hp in range(4, 8)],
                         do3)
            elif do3:
                for b in range(B):
                    stage_p3(b)

    nc.compile()
    return nc


def make_in_maps(x, in_proj_weight, in_proj_bias, out_proj_weight,
                 out_proj_bias):
    f32, bf16 = np.float32, np.float16
    x = np.asarray(x, f32)
    fp8 = ml_dtypes.float8_e4m3
    wiT = np.asarray(in_proj_weight, f32).T              # [D, 3D]
    # [hp*128+p, dc, c] layouts so each weight loads in one DMA
    wq8 = np.ascontiguousarray(
        (wiT[:, :D] * 4096.0).astype(fp8).reshape(8, 128, 8, 128)
        .transpose(2, 1, 0, 3).reshape(D, 8, 128))
    wk8 = np.ascontiguousarray(
        (wiT[:, D:2 * D] * 4096.0).astype(fp8).reshape(8, 128, 8, 128)
        .transpose(2, 1, 0, 3).reshape(D, 8, 128))
    wkT = np.ascontiguousarray(
        wiT[:, D:2 * D].astype(bf16).reshape(8, 128, 8, 128)
        .transpose(2, 1, 0, 3).reshape(D, 8, 128))
    wvT = np.ascontiguousarray(
        wiT[:, 2 * D:].astype(bf16).reshape(8, 128, 2, 512)
        .transpose(2, 1, 0, 3).reshape(256, 8, 512))
    bi = np.asarray(in_proj_bias, f32)
    biT = np.ascontiguousarray(bi[:2 * D].reshape(16, 128).T)
    wo = np.asarray(out_proj_weight, f32)
    woT = np.ascontiguousarray(
        wo.T.astype(bf16).reshape(8, 128, D).transpose(1, 0, 2))
    bop = wo @ bi[2 * D:] + np.asarray(out_proj_bias, f32)
    boT = np.ascontiguousarray(bop.reshape(8, 128).T)

    p = np.arange(128)
    cstN = np.zeros((128, 192), f32)
    cstN[:, 0:128] = ((p[:, None] < 64) == (p[None, :] < 64))
    cstN[:, 128:192] = 1.0
    cstN = cstN.astype(bf16)

    k2 = np.arange(128)[:, None]
    q = np.arange(SC)[None, :]
    mloc = (((k2 // 64) == ((q // 64) % 2)) & ((q % 64) >= (k2 % 64)))
    mloc2 = np.tile(mloc.astype(f32), (1, 2))

    xs = x[:, BLK - 1::BLK, :]                           # [B, 64, D]
    xsTf = xs.transpose(2, 0, 1).reshape(D, NSUM)
    xsT = np.ascontiguousarray(
        xsTf.astype(bf16).reshape(8, 128, NSUM).transpose(1, 0, 2))
    xs8 = np.ascontiguousarray(
        (xsTf * 16.0).astype(fp8).reshape(8, 128, NSUM).transpose(1, 0, 2))

    m = np.arange(64)[:, None]
    in_maps = []
    for c in range(NCORES):
        xc = x[:, c * SC:(c + 1) * SC, :]                # [B, 512, D]
        xTf = xc.transpose(2, 0, 1).reshape(D, TC)
        xTc = np.ascontiguousarray(
            xTf.astype(bf16).reshape(8, 128, TC).transpose(1, 0, 2))
        xT8c = np.ascontiguousarray(
            (xTf * 16.0).astype(fp8).reshape(8, 128, TC).transpose(1, 0, 2))
        ms = (m < (c * BPC + (q // 64))).astype(f32)     # [64, 512]
        mcomb = np.concatenate(
            [mloc2, np.concatenate([ms, ms], 0)], 1).astype(bf16)
        in_maps.append({
            "xT": xTc, "xT8": xT8c, "xsT": xsT, "xs8": xs8, "wq8": wq8,
            "wk8": wk8, "wkT": wkT, "wvT": wvT, "biT": biT,
            "woT": woT, "boT": boT, "cstN": cstN, "mcomb": mcomb,
        })
    return in_maps


_NC_CACHE = []


def kernel(x, in_proj_weight, in_proj_bias, out_proj_weight, out_proj_bias):
    if not _NC_CACHE:
        _NC_CACHE.append(build_nc())
    nc = _NC_CACHE[0]
    in_maps = make_in_maps(x, in_proj_weight, in_proj_bias, out_proj_weight,
                           out_proj_bias)
    res = run_bass_kernel_spmd(nc, in_maps, core_ids=list(range(NCORES)))
    out = np.empty((B, S, D), np.float32)
    for c in range(NCORES):
        oT = np.asarray(res.results[c]["outT"]).astype(np.float32)
        out[:, c * SC:(c + 1) * SC, :] = \
            oT.reshape(D, B, SC).transpose(1, 2, 0)
    return out


# revision 20
# speedup vs baseline: 1.1735x; 1.1735x over previous
"""CronRoot (sqrt-N block-sparse causal) multihead attention on 8 trn2 cores.

v3 (from v2 base): k-projection moves to fp8 DoubleRow like q (measured
rel-err 1.84e-2 vs the 2e-2 gate, verified against a bit-matched numpy
emulation of the kernel numerics); softmax denominator is computed with
ones-block stationaries that broadcast l directly to [128,512] PSUM,
dropping the separate broadcast matmul, the [2,512] reciprocal and the
av PSUM->SBUF staging copy; the three phases share one pool scope so the
tile scheduler overlaps wave-1 attention with the second half of the
projections; DMA instruction count cut ~5x by host-side re-layout of
weights/x so each tensor loads with one large-descriptor DMA.

Sharding: sequence-parallel. Each core owns 8 of the 64 blocks (512
positions) for all batches/heads; summary k/v recomputed per-core from the
256 summary rows of x (no collectives).

Engine split per (b, head-pair) attention instance:
  PE: 8 local-score mm, 1 summary-score mm (block-diag ksd), 3 denominator
      mm (broadcast-l), 9 AV mm (block-diag vsd + 8 local).
  Scalar: exp(local [128,1024]), exp(summary [128,512]).
  GpSimd: local mask multiply. DVE: summary mask multiply, reciprocal
  [128,512], final (av*1/l) -> bf16 attnT.
"""

import numpy as np
import ml_dtypes
from contextlib import ExitStack

import concourse.bass as bass  # noqa: F401
import concourse.tile as tile
from concourse import bacc, mybir
from concourse.bass_utils import run_bass_kernel_spmd

F32 = mybir.dt.float32
FP8 = mybir.dt.float8e4
DR = mybir.MatmulPerfMode.DoubleRow
QSC = 1.0 / 65536.0  # undo x*16 and w*4096 scaling
FP16 = mybir.dt.bfloat16  # fp16 matmuls measured ~2x slower on HW
AF = mybir.ActivationFunctionType

B, S, D = 4, 4096, 1024
H, HD = 16, 64
BLK = 64                 # block size (= sqrt(S))
NB = S // BLK            # 64 blocks
NCORES = 8
SC = S // NCORES         # 512 seq positions per core
BPC = NB // NCORES       # 8 blocks per core
TC = B * SC              # 2048 (b-major) t columns per core
NSUM = B * NB            # 256 summary positions (b-major)
SCALE = 1.0 / np.sqrt(HD)


def build_nc(repeat=1, phases=(1, 2, 3), k_fp8=True):
    nc = bacc.Bacc("TRN2", target_bir_lowering=False, debug=False,
                   num_devices=NCORES)

    xT = nc.dram_tensor("xT", [128, 8, TC], FP16, kind="ExternalInput").ap()
    xT8 = nc.dram_tensor("xT8", [128, 8, TC], FP8, kind="ExternalInput").ap()
    xsT = nc.dram_tensor("xsT", [128, 8, NSUM], FP16,
                         kind="ExternalInput").ap()
    xs8 = nc.dram_tensor("xs8", [128, 8, NSUM], FP8,
                         kind="ExternalInput").ap()
    wq8 = nc.dram_tensor("wq8", [D, 8, 128], FP8, kind="ExternalInput").ap()
    wk8 = nc.dram_tensor("wk8", [D, 8, 128], FP8, kind="ExternalInput").ap()
    wkT = nc.dram_tensor("wkT", [D, 8, 128], FP16, kind="ExternalInput").ap()
    wvT = nc.dram_tensor("wvT", [256, 8, 512], FP16,
                         kind="ExternalInput").ap()
    biT = nc.dram_tensor("biT", [128, 16], F32, kind="ExternalInput").ap()
    woT = nc.dram_tensor("woT", [128, 8, D], FP16, kind="ExternalInput").ap()
    boT = nc.dram_tensor("boT", [128, 8], F32, kind="ExternalInput").ap()
    cstN = nc.dram_tensor("cstN", [128, 192], FP16, kind="ExternalInput").ap()
    mcomb = nc.dram_tensor("mcomb", [128, 1536], FP16,
                           kind="ExternalInput").ap()
    outT = nc.dram_tensor("outT", [D, TC], FP16, kind="ExternalOutput").ap()

    with tile.TileContext(nc) as tc_:
      for _rep in range(repeat):
       with ExitStack() as ctx:
        pp = ctx.enter_context(tc_.tile_pool(name="persist", bufs=1))
        qT = pp.tile([128, 8, TC], FP16, tag="qT")
        kT = pp.tile([128, 8, TC], FP16, tag="kT")
        v_sb = pp.tile([128, 16, 16, HD], FP16, tag="v")     # (tcn, h, d)
        ksd = pp.tile([128, 32, 128], FP16, tag="ksd")       # (hp*4+b) diag
        vsd = pp.tile([128, 32, 128], FP16, tag="vsd")
        attnT = pp.tile([128, 8, TC], FP16, tag="attnT")

        pw = ctx.enter_context(tc_.tile_pool(name="wstage", bufs=2))
        p2 = ctx.enter_context(tc_.tile_pool(name="p2", bufs=2))
        psq = ctx.enter_context(tc_.tile_pool(name="ps", bufs=1,
                                              space="PSUM"))
        px_cm = tc_.tile_pool(name="xstage", bufs=1)
        px = px_cm.__enter__()

        # critical-path loads first: x chunks feed the first q/k matmuls;
        # split across the SP and Activation HWDGE queues to halve latency
        xT8_sb = px.tile([128, 8, TC], FP8, tag="xT8")
        for dc in range(4):
            eng = nc.sync if dc % 2 == 0 else nc.scalar
            eng.dma_start(xT8_sb[:, 2 * dc:2 * dc + 2, :],
                          xT8[:, 2 * dc:2 * dc + 2, :])
        biT_sb = pp.tile([128, 16], F32, tag="biT")
        nc.sync.dma_start(biT_sb[:], biT[:])
        xsT_sb = px.tile([128, 8, NSUM], FP16, tag="xsT")
        nc.scalar.dma_start(xsT_sb[:], xsT[:])
        xs8_sb = px.tile([128, 8, NSUM], FP8, tag="xs8")
        nc.scalar.dma_start(xs8_sb[:], xs8[:])
        boT_sb = pp.tile([128, 8], F32, tag="boT")
        cst_sb = pp.tile([128, 192], FP16, tag="cstN")
        mcomb_sb = pp.tile([128, 1536], FP16, tag="mcomb")
        nc.vector.memset(ksd[:], 0.0)
        nc.vector.memset(vsd[:], 0.0)

        # ---------------- P1 projections for one half ----------------
        def p1_half(vh):
            for hp in range(4 * vh, 4 * vh + 4):
                # q chunk: fp8 DoubleRow (x*16, wq*4096)
                wq_sb = pw.tile([128, 8, 128], FP8, tag="w8_sb")
                nc.sync.dma_start(wq_sb[:],
                                  wq8[hp * 128:(hp + 1) * 128, :, :])
                for tt in range(4):
                    ps_qk = psq.tile([128, 512], F32, tag="p512", bufs=3)
                    for kp in range(4):
                        nc.tensor.matmul(
                            ps_qk[:],
                            wq_sb[:, 2 * kp:2 * kp + 2, :],
                            xT8_sb[:, 2 * kp:2 * kp + 2,
                                   tt * 512:(tt + 1) * 512],
                            start=(kp == 0), stop=(kp == 3),
                            perf_mode=DR)
                    nc.vector.tensor_scalar(
                        qT[:, hp, tt * 512:(tt + 1) * 512], ps_qk[:],
                        QSC, biT_sb[:, hp:hp + 1],
                        mybir.AluOpType.mult, mybir.AluOpType.add)
                # k chunk (+ summary keys into block-diag ksd)
                if k_fp8:
                    wk_sb = pw.tile([128, 8, 128], FP8, tag="wk_sb")
                    nc.sync.dma_start(wk_sb[:],
                                      wk8[hp * 128:(hp + 1) * 128, :, :])
                    for tt in range(4):
                        ps_qk = psq.tile([128, 512], F32, tag="p512", bufs=3)
                        for kp in range(4):
                            nc.tensor.matmul(
                                ps_qk[:],
                                wk_sb[:, 2 * kp:2 * kp + 2, :],
                                xT8_sb[:, 2 * kp:2 * kp + 2,
                                       tt * 512:(tt + 1) * 512],
                                start=(kp == 0), stop=(kp == 3),
                                perf_mode=DR)
                        nc.gpsimd.tensor_scalar(
                            kT[:, hp, tt * 512:(tt + 1) * 512], ps_qk[:],
                            QSC, biT_sb[:, hp + 8:hp + 9],
                            mybir.AluOpType.mult, mybir.AluOpType.add)
                    ps_ks = psq.tile([128, 4, 64], F32, tag="p512", bufs=3)
                    for kp in range(4):
                        nc.tensor.matmul(
                            ps_ks[:],
                            wk_sb[:, 2 * kp:2 * kp + 2, :],
                            xs8_sb[:, 2 * kp:2 * kp + 2, :],
                            start=(kp == 0), stop=(kp == 3),
                            perf_mode=DR)
                    ksc = QSC
                else:
                    wk_sb = pw.tile([128, 8, 128], FP16, tag="wk_sb")
                    nc.sync.dma_start(wk_sb[:],
                                      wkT[hp * 128:(hp + 1) * 128, :, :])
                    for tt in range(4):
                        kx = pw.tile([128, 8, 512], FP16, tag="kx")
                        nc.sync.dma_start(kx[:],
                                          xT[:, :, tt * 512:(tt + 1) * 512])
                        ps_qk = psq.tile([128, 512], F32, tag="p512", bufs=3)
                        for dc in range(8):
                            nc.tensor.matmul(
                                ps_qk[:], wk_sb[:, dc, :],
                                kx[:, dc, :],
                                start=(dc == 0), stop=(dc == 7))
                        nc.scalar.activation(
                            kT[:, hp, tt * 512:(tt + 1) * 512], ps_qk[:],
                            AF.Identity, bias=biT_sb[:, hp + 8:hp + 9])
                    ps_ks = psq.tile([128, 4, 64], F32, tag="p512", bufs=3)
                    for dc in range(8):
                        nc.tensor.matmul(ps_ks[:], wk_sb[:, dc, :],
                                         xsT_sb[:, dc, :],
                                         start=(dc == 0), stop=(dc == 7))
                    ksc = 1.0
                nc.scalar.activation(
                    ksd[0:64, hp * 4:hp * 4 + 4, 0:64], ps_ks[0:64],
                    AF.Identity, scale=ksc, bias=biT_sb[0:64, hp + 8:hp + 9])
                nc.scalar.activation(
                    ksd[64:128, hp * 4:hp * 4 + 4, 64:128], ps_ks[64:128],
                    AF.Identity, scale=ksc,
                    bias=biT_sb[64:128, hp + 8:hp + 9])

            # v features for this half (heads 8*vh .. 8*vh+8)
            wv_sb = pw.tile([128, 8, 512], FP16, tag="wv_sb", bufs=1)
            nc.sync.dma_start(wv_sb[:], wvT[vh * 128:(vh + 1) * 128, :, :])
            for tcn in range(16):
                xch = pw.tile([128, 8, 128], FP16, tag="xch", bufs=3)
                nc.sync.dma_start(xch[:],
                                  xT[:, :, tcn * 128:(tcn + 1) * 128])
                ps_v = psq.tile([128, 512], F32, tag="p512", bufs=3)
                for dc in range(8):
                    nc.tensor.matmul(
                        ps_v[:],
                        xch[:, dc, :],
                        wv_sb[:, dc, :],
                        start=(dc == 0), stop=(dc == 7))
                nc.scalar.copy(
                    v_sb[:, tcn, vh * 8:(vh + 1) * 8, :], ps_v[:])
            # summary v -> vs2 (with duplicated partition halves)
            vs2 = px.tile([128, 4, 8, HD], FP16, tag="vs2")
            for sch in range(2):
                ps_vs = psq.tile([128, 512], F32, tag="p512", bufs=3)
                for dc in range(8):
                    nc.tensor.matmul(
                        ps_vs[:],
                        xsT_sb[:, dc, sch * 128:(sch + 1) * 128],
                        wv_sb[:, dc, :],
                        start=(dc == 0), stop=(dc == 7))
                nc.scalar.copy(vs2[0:64, 2 * sch, :, :], ps_vs[0:64])
                nc.scalar.copy(vs2[64:128, 2 * sch + 1, :, :],
                               ps_vs[64:128])
                nc.sync.dma_start(vs2[64:128, 2 * sch, :, :],
                                  vs2[0:64, 2 * sch, :, :])
                nc.sync.dma_start(vs2[0:64, 2 * sch + 1, :, :],
                                  vs2[64:128, 2 * sch + 1, :, :])
            # block-diag summary-v stationaries for this vh's head pairs
            for hp in range(4 * vh, 4 * vh + 4):
                hl = 2 * hp - 8 * vh        # head index within vs2 cols
                nc.vector.tensor_copy(
                    vsd[0:64, hp * 4:hp * 4 + 4, 0:64],
                    vs2[0:64, :, hl, :])
                nc.vector.tensor_copy(
                    vsd[64:128, hp * 4:hp * 4 + 4, 64:128],
                    vs2[64:128, :, hl + 1, :])

        # ------------- P2 attention stages -------------
        def stage_scores(b, hp):
            c0 = b * 512
            s_loc = psq.tile([128, 1024], F32, tag="u1024", bufs=1)
            for hh in range(2):
                for p4 in range(4):
                    cq = c0 + p4 * 128
                    nc.tensor.matmul(
                        s_loc[:, hh * 512 + p4 * 128:
                              hh * 512 + (p4 + 1) * 128],
                        kT[hh * 64:hh * 64 + 64, hp, cq:cq + 128],
                        qT[hh * 64:hh * 64 + 64, hp, cq:cq + 128],
                        start=True, stop=True)
            s_sum = psq.tile([128, 512], F32, tag="u512", bufs=3)
            nc.tensor.matmul(s_sum[:], ksd[:, hp * 4 + b, :],
                             qT[:, hp, c0:c0 + 512],
                             start=True, stop=True)
            pml = p2.tile([128, 1024], FP16, tag="pml")
            nc.scalar.activation(pml[:], s_loc[:], AF.Exp, scale=SCALE)
            pms = p2.tile([128, 512], FP16, tag="pms")
            nc.scalar.activation(pms[:], s_sum[:], AF.Exp, scale=SCALE)
            nc.gpsimd.tensor_mul(pml[:], pml[:], mcomb_sb[:, 0:1024])
            nc.vector.tensor_mul(pms[:], pms[:], mcomb_sb[:, 1024:1536])
            return pml, pms

        def stage_av(b, hp, pml, pms):
            c0 = b * 512
            l_bc = psq.tile([128, 512], F32, tag="u512", bufs=3)
            nc.tensor.matmul(l_bc[:], cst_sb[:, 0:128], pms[:],
                             start=True, stop=False, skip_group_check=True)
            nc.tensor.matmul(l_bc[0:64, :], cst_sb[:, 128:192],
                             pml[:, 0:512],
                             start=False, stop=False, skip_group_check=True)
            nc.tensor.matmul(l_bc[64:128, :], cst_sb[:, 128:192],
                             pml[:, 512:1024],
                             start=False, stop=True, skip_group_check=True)
            rinv = p2.tile([128, 512], FP16, tag="rinv")
            with nc.allow_low_precision(reason="fp16 softmax recip"):
                nc.vector.reciprocal(rinv[:], l_bc[:])
            av = psq.tile([128, 512], F32, tag="u512", bufs=3)
            nc.tensor.matmul(av[:], vsd[:, hp * 4 + b, :], pms[:],
                             start=True, stop=False, skip_group_check=True)
            for hh in range(2):
                for p4 in range(4):
                    nc.tensor.matmul(
                        av[hh * 64:hh * 64 + 64, p4 * 128:(p4 + 1) * 128],
                        v_sb[:, b * 4 + p4, 2 * hp + hh, :],
                        pml[:, hh * 512 + p4 * 128:
                            hh * 512 + (p4 + 1) * 128],
                        start=False, stop=(p4 == 3),
                        skip_group_check=True)
            nc.vector.tensor_mul(attnT[:, hp, c0:c0 + 512], av[:], rinv[:])

        def stage_p3(b):
            c0 = b * 512
            for oc in range(8):
                ps_o = psq.tile([128, 512], F32, tag="p512", bufs=3)
                for hp in range(8):
                    nc.tensor.matmul(
                        ps_o[:],
                        woT_sb[:, hp, oc * 128:(oc + 1) * 128],
                        attnT[:, hp, c0:c0 + 512],
                        start=(hp == 0), stop=(hp == 7))
                o_sb = p2.tile([128, 512], FP16, tag="o_sb", bufs=2)
                nc.vector.tensor_scalar(o_sb[:], ps_o[:],
                                        boT_sb[:, oc:oc + 1], None,
                                        mybir.AluOpType.add)
                nc.sync.dma_start(
                    outT[oc * 128:(oc + 1) * 128, c0:c0 + 512], o_sb[:])

        def run_wave(pairs, with_p3):
            pend = stage_scores(*pairs[0])
            for i, (b, hp) in enumerate(pairs):
                if i + 1 < len(pairs):
                    nxt = stage_scores(*pairs[i + 1])
                else:
                    nxt = None
                stage_av(b, hp, *pend)
                pend = nxt
                if with_p3 and hp == 7:
                    stage_p3(b)

        do2 = 2 in phases
        do3 = 3 in phases
        if 1 in phases:
            p1_half(0)
            nc.sync.dma_start(cst_sb[:], cstN[:])
            nc.sync.dma_start(mcomb_sb[:], mcomb[:])
            nc.sync.dma_start(boT_sb[:], boT[:])
            if do2:
                run_wave([(b, hp) for b in range(B) for hp in range(4)],
                         False)
            p1_half(1)
            px_cm.__exit__(None, None, None)
            pl = ctx.enter_context(tc_.tile_pool(name="late", bufs=1))
            woT_sb = pl.tile([128, 8, D], FP16, tag="woT")
            nc.sync.dma_start(woT_sb[:], woT[:])
            if do2:
                run_wave([(b, hp) for b in range(B) for hp in range(4, 8)],
                         do3)
            elif do3:
                for b in range(B):
                    stage_p3(b)

    nc.compile()
    return nc


def make_in_maps(x, in_proj_weight, in_proj_bias, out_proj_weight,
                 out_proj_bias):
    f32, bf16 = np.float32, ml_dtypes.bfloat16
    x = np.asarray(x, f32)
    fp8 = ml_dtypes.float8_e4m3
    wiT = np.asarray(in_proj_weight, f32).T              # [D, 3D]
    # [hp*128+p, dc, c] layouts so each weight loads in one DMA
    wq8 = np.ascontiguousarray(
        (wiT[:, :D] * 4096.0).astype(fp8).reshape(8, 128, 8, 128)
        .transpose(2, 1, 0, 3).reshape(D, 8, 128))
    wk8 = np.ascontiguousarray(
        (wiT[:, D:2 * D] * 4096.0).astype(fp8).reshape(8, 128, 8, 128)
        .transpose(2, 1, 0, 3).reshape(D, 8, 128))
    wkT = np.ascontiguousarray(
        wiT[:, D:2 * D].astype(bf16).reshape(8, 128, 8, 128)
        .transpose(2, 1, 0, 3).reshape(D, 8, 128))
    wvT = np.ascontiguousarray(
        wiT[:, 2 * D:].astype(bf16).reshape(8, 128, 2, 512)
        .transpose(2, 1, 0, 3).reshape(256, 8, 512))
    bi = np.asarray(in_proj_bias, f32)
    biT = np.ascontiguousarray(bi[:2 * D].reshape(16, 128).T)
    wo = np.asarray(out_proj_weight, f32)
    woT = np.ascontiguousarray(
        wo.T.astype(bf16).reshape(8, 128, D).transpose(1, 0, 2))
    bop = wo @ bi[2 * D:] + np.asarray(out_proj_bias, f32)
    boT = np.ascontiguousarray(bop.reshape(8, 128).T)

    p = np.arange(128)
    cstN = np.zeros((128, 192), f32)
    cstN[:, 0:128] = ((p[:, None] < 64) == (p[None, :] < 64))
    cstN[:, 128:192] = 1.0
    cstN = cstN.astype(bf16)

    k2 = np.arange(128)[:, None]
    q = np.arange(SC)[None, :]
    mloc = (((k2 // 64) == ((q // 64) % 2)) & ((q % 64) >= (k2 % 64)))
    mloc2 = np.tile(mloc.astype(f32), (1, 2))

    xs = x[:, BLK - 1::BLK, :]                           # [B, 64, D]
    xsTf = xs.transpose(2, 0, 1).reshape(D, NSUM)
    xsT = np.ascontiguousarray(
        xsTf.astype(bf16).reshape(8, 128, NSUM).transpose(1, 0, 2))
    xs8 = np.ascontiguousarray(
        (xsTf * 16.0).astype(fp8).reshape(8, 128, NSUM).transpose(1, 0, 2))

    m = np.arange(64)[:, None]
    in_maps = []
    for c in range(NCORES):
        xc = x[:, c * SC:(c + 1) * SC, :]                # [B, 512, D]
        xTf = xc.transpose(2, 0, 1).reshape(D, TC)
        xTc = np.ascontiguousarray(
            xTf.astype(bf16).reshape(8, 128, TC).transpose(1, 0, 2))
        xT8c = np.ascontiguousarray(
            (xTf * 16.0).astype(fp8).reshape(8, 128, TC).transpose(1, 0, 2))
        ms = (m < (c * BPC + (q // 64))).astype(f32)     # [64, 512]
        mcomb = np.concatenate(
            [mloc2, np.concatenate([ms, ms], 0)], 1).astype(bf16)
        in_maps.append({
            "xT": xTc, "xT8": xT8c, "xsT": xsT, "xs8": xs8, "wq8": wq8,
            "wk8": wk8, "wkT": wkT, "wvT": wvT, "biT": biT,
            "woT": woT, "boT": boT, "cstN": cstN, "mcomb": mcomb,
        })
    return in_maps


_NC_CACHE = []


def kernel(x, in_proj_weight, in_proj_bias, out_proj_weight, out_proj_bias):
    if not _NC_CACHE:
        _NC_CACHE.append(build_nc())
    nc = _NC_CACHE[0]
    in_maps = make_in_maps(x, in_proj_weight, in_proj_bias, out_proj_weight,
                           out_proj_bias)
    res = run_bass_kernel_spmd(nc, in_maps, core_ids=list(range(NCORES)))
    out = np.empty((B, S, D), np.float32)
    for c in range(NCORES):
        oT = np.asarray(res.results[c]["outT"]).astype(np.float32)
        out[:, c * SC:(c + 1) * SC, :] = \
            oT.reshape(D, B, SC).transpose(1, 2, 0)
    return out


# revision 21
# speedup vs baseline: 1.2057x; 1.0274x over previous
"""CronRoot (sqrt-N block-sparse causal) multihead attention on 8 trn2 cores.

v3 (from v2 base): k-projection moves to fp8 DoubleRow like q (measured
rel-err 1.84e-2 vs the 2e-2 gate, verified against a bit-matched numpy
emulation of the kernel numerics); softmax denominator is computed with
ones-block stationaries that broadcast l directly to [128,512] PSUM,
dropping the separate broadcast matmul, the [2,512] reciprocal and the
av PSUM->SBUF staging copy; the three phases share one pool scope so the
tile scheduler overlaps wave-1 attention with the second half of the
projections; DMA instruction count cut ~5x by host-side re-layout of
weights/x so each tensor loads with one large-descriptor DMA.

Sharding: sequence-parallel. Each core owns 8 of the 64 blocks (512
positions) for all batches/heads; summary k/v recomputed per-core from the
256 summary rows of x (no collectives).

Engine split per (b, head-pair) attention instance:
  PE: 8 local-score mm, 1 summary-score mm (block-diag ksd), 3 denominator
      mm (broadcast-l), 9 AV mm (block-diag vsd + 8 local).
  Scalar: exp(local [128,1024]), exp(summary [128,512]).
  GpSimd: local mask multiply. DVE: summary mask multiply, reciprocal
  [128,512], final (av*1/l) -> bf16 attnT.
"""

import numpy as np
import ml_dtypes
from contextlib import ExitStack

import concourse.bass as bass  # noqa: F401
import concourse.tile as tile
from concourse import bacc, mybir
from concourse.bass_utils import run_bass_kernel_spmd

F32 = mybir.dt.float32
FP8 = mybir.dt.float8e4
DR = mybir.MatmulPerfMode.DoubleRow
QSC = 1.0 / 65536.0  # undo x*16 and w*4096 scaling
BF16 = mybir.dt.bfloat16
AF = mybir.ActivationFunctionType

B, S, D = 4, 4096, 1024
H, HD = 16, 64
BLK = 64                 # block size (= sqrt(S))
NB = S // BLK            # 64 blocks
NCORES = 8
SC = S // NCORES         # 512 seq positions per core
BPC = NB // NCORES       # 8 blocks per core
TC = B * SC              # 2048 (b-major) t columns per core
NSUM = B * NB            # 256 summary positions (b-major)
SCALE = 1.0 / np.sqrt(HD)


def build_nc(repeat=1, phases=(1, 2, 3), k_fp8=True):
    nc = bacc.Bacc("TRN2", target_bir_lowering=False, debug=False,
                   num_devices=NCORES)

    xT = nc.dram_tensor("xT", [128, 8, TC], BF16, kind="ExternalInput").ap()
    xT8 = nc.dram_tensor("xT8", [128, 8, TC], FP8, kind="ExternalInput").ap()
    xsT = nc.dram_tensor("xsT", [128, 8, NSUM], BF16,
                         kind="ExternalInput").ap()
    xs8 = nc.dram_tensor("xs8", [128, 8, NSUM], FP8,
                         kind="ExternalInput").ap()
    wq8 = nc.dram_tensor("wq8", [D, 8, 128], FP8, kind="ExternalInput").ap()
    wk8 = nc.dram_tensor("wk8", [D, 8, 128], FP8, kind="ExternalInput").ap()
    wkT = nc.dram_tensor("wkT", [D, 8, 128], BF16, kind="ExternalInput").ap()
    wvT = nc.dram_tensor("wvT", [256, 8, 512], BF16,
                         kind="ExternalInput").ap()
    biT = nc.dram_tensor("biT", [128, 16], F32, kind="ExternalInput").ap()
    woT = nc.dram_tensor("woT", [128, 8, D], BF16, kind="ExternalInput").ap()
    boT = nc.dram_tensor("boT", [128, 8], F32, kind="ExternalInput").ap()
    cstN = nc.dram_tensor("cstN", [128, 192], BF16, kind="ExternalInput").ap()
    mloc2 = nc.dram_tensor("mloc2", [128, 1024], BF16,
                           kind="ExternalInput").ap()
    msum2 = nc.dram_tensor("msum2", [128, SC], BF16,
                           kind="ExternalInput").ap()
    outT = nc.dram_tensor("outT", [D, TC], BF16, kind="ExternalOutput").ap()

    with tile.TileContext(nc) as tc_:
      for _rep in range(repeat):
       with ExitStack() as ctx:
        pp = ctx.enter_context(tc_.tile_pool(name="persist", bufs=1))
        qT = pp.tile([128, 8, TC], BF16, tag="qT")
        kT = pp.tile([128, 8, TC], BF16, tag="kT")
        v_sb = pp.tile([128, 16, 16, HD], BF16, tag="v")     # (tcn, h, d)
        ksd = pp.tile([128, 32, 128], BF16, tag="ksd")       # (hp*4+b) diag
        vsd = pp.tile([128, 32, 128], BF16, tag="vsd")
        attnT = pp.tile([128, 8, TC], BF16, tag="attnT")
        biT_sb = pp.tile([128, 16], F32, tag="biT")
        nc.sync.dma_start(biT_sb[:], biT[:])
        boT_sb = pp.tile([128, 8], F32, tag="boT")
        nc.sync.dma_start(boT_sb[:], boT[:])
        cst_sb = pp.tile([128, 192], BF16, tag="cstN")
        nc.sync.dma_start(cst_sb[:], cstN[:])
        mloc2_sb = pp.tile([128, 1024], BF16, tag="mloc2")
        nc.sync.dma_start(mloc2_sb[:], mloc2[:])
        msum2_sb = pp.tile([128, SC], BF16, tag="msum2")
        nc.sync.dma_start(msum2_sb[:], msum2[:])
        nc.vector.memset(ksd[:], 0.0)
        nc.vector.memset(vsd[:], 0.0)

        pw = ctx.enter_context(tc_.tile_pool(name="wstage", bufs=2))
        p2 = ctx.enter_context(tc_.tile_pool(name="p2", bufs=2))
        psq = ctx.enter_context(tc_.tile_pool(name="ps", bufs=1,
                                              space="PSUM"))
        px_cm = tc_.tile_pool(name="xstage", bufs=1)
        px = px_cm.__enter__()

        xT8_sb = px.tile([128, 8, TC], FP8, tag="xT8")
        for dc in range(4):
            nc.sync.dma_start(xT8_sb[:, 2 * dc:2 * dc + 2, :],
                              xT8[:, 2 * dc:2 * dc + 2, :])
        xsT_sb = px.tile([128, 8, NSUM], BF16, tag="xsT")
        nc.sync.dma_start(xsT_sb[:], xsT[:])
        xs8_sb = px.tile([128, 8, NSUM], FP8, tag="xs8")
        nc.sync.dma_start(xs8_sb[:], xs8[:])

        # ---------------- P1 projections for one half ----------------
        def p1_half(vh):
            for hp in range(4 * vh, 4 * vh + 4):
                # q chunk: fp8 DoubleRow (x*16, wq*4096)
                wq_sb = pw.tile([128, 8, 128], FP8, tag="w8_sb")
                nc.sync.dma_start(wq_sb[:],
                                  wq8[hp * 128:(hp + 1) * 128, :, :])
                for tt in range(4):
                    ps_qk = psq.tile([128, 512], F32, tag="p512", bufs=3)
                    for kp in range(4):
                        nc.tensor.matmul(
                            ps_qk[:],
                            wq_sb[:, 2 * kp:2 * kp + 2, :],
                            xT8_sb[:, 2 * kp:2 * kp + 2,
                                   tt * 512:(tt + 1) * 512],
                            start=(kp == 0), stop=(kp == 3),
                            perf_mode=DR)
                    nc.scalar.activation(
                        qT[:, hp, tt * 512:(tt + 1) * 512], ps_qk[:],
                        AF.Identity, scale=QSC,
                        bias=biT_sb[:, hp:hp + 1])
                # k chunk (+ summary keys into block-diag ksd)
                if k_fp8:
                    wk_sb = pw.tile([128, 8, 128], FP8, tag="wk_sb")
                    nc.sync.dma_start(wk_sb[:],
                                      wk8[hp * 128:(hp + 1) * 128, :, :])
                    for tt in range(4):
                        ps_qk = psq.tile([128, 512], F32, tag="p512", bufs=3)
                        for kp in range(4):
                            nc.tensor.matmul(
                                ps_qk[:],
                                wk_sb[:, 2 * kp:2 * kp + 2, :],
                                xT8_sb[:, 2 * kp:2 * kp + 2,
                                       tt * 512:(tt + 1) * 512],
                                start=(kp == 0), stop=(kp == 3),
                                perf_mode=DR)
                        nc.scalar.activation(
                            kT[:, hp, tt * 512:(tt + 1) * 512], ps_qk[:],
                            AF.Identity, scale=QSC,
                            bias=biT_sb[:, hp + 8:hp + 9])
                    ps_ks = psq.tile([128, 4, 64], F32, tag="p512", bufs=3)
                    for kp in range(4):
                        nc.tensor.matmul(
                            ps_ks[:],
                            wk_sb[:, 2 * kp:2 * kp + 2, :],
                            xs8_sb[:, 2 * kp:2 * kp + 2, :],
                            start=(kp == 0), stop=(kp == 3),
                            perf_mode=DR)
                    ksc = QSC
                else:
                    wk_sb = pw.tile([128, 8, 128], BF16, tag="wk_sb")
                    nc.sync.dma_start(wk_sb[:],
                                      wkT[hp * 128:(hp + 1) * 128, :, :])
                    for tt in range(4):
                        kx = pw.tile([128, 8, 512], BF16, tag="kx")
                        nc.sync.dma_start(kx[:],
                                          xT[:, :, tt * 512:(tt + 1) * 512])
                        ps_qk = psq.tile([128, 512], F32, tag="p512", bufs=3)
                        for dc in range(8):
                            nc.tensor.matmul(
                                ps_qk[:], wk_sb[:, dc, :],
                                kx[:, dc, :],
                                start=(dc == 0), stop=(dc == 7))
                        nc.scalar.activation(
                            kT[:, hp, tt * 512:(tt + 1) * 512], ps_qk[:],
                            AF.Identity, bias=biT_sb[:, hp + 8:hp + 9])
                    ps_ks = psq.tile([128, 4, 64], F32, tag="p512", bufs=3)
                    for dc in range(8):
                        nc.tensor.matmul(ps_ks[:], wk_sb[:, dc, :],
                                         xsT_sb[:, dc, :],
                                         start=(dc == 0), stop=(dc == 7))
                    ksc = 1.0
                nc.scalar.activation(
                    ksd[0:64, hp * 4:hp * 4 + 4, 0:64], ps_ks[0:64],
                    AF.Identity, scale=ksc, bias=biT_sb[0:64, hp + 8:hp + 9])
                nc.scalar.activation(
                    ksd[64:128, hp * 4:hp * 4 + 4, 64:128], ps_ks[64:128],
                    AF.Identity, scale=ksc,
                    bias=biT_sb[64:128, hp + 8:hp + 9])

            # v features for this half (heads 8*vh .. 8*vh+8)
            wv_sb = pw.tile([128, 8, 512], BF16, tag="wv_sb", bufs=1)
            nc.sync.dma_start(wv_sb[:], wvT[vh * 128:(vh + 1) * 128, :, :])
            for tcn in range(16):
                xch = pw.tile([128, 8, 128], BF16, tag="xch", bufs=3)
                nc.sync.dma_start(xch[:],
                                  xT[:, :, tcn * 128:(tcn + 1) * 128])
                ps_v = psq.tile([128, 512], F32, tag="p512", bufs=3)
                for dc in range(8):
                    nc.tensor.matmul(
                        ps_v[:],
                        xch[:, dc, :],
                        wv_sb[:, dc, :],
                        start=(dc == 0), stop=(dc == 7))
                nc.scalar.copy(
                    v_sb[:, tcn, vh * 8:(vh + 1) * 8, :], ps_v[:])
            # summary v -> vs2 (with duplicated partition halves)
            vs2 = px.tile([128, 4, 8, HD], BF16, tag="vs2")
            for sch in range(2):
                ps_vs = psq.tile([128, 512], F32, tag="p512", bufs=3)
                for dc in range(8):
                    nc.tensor.matmul(
                        ps_vs[:],
                        xsT_sb[:, dc, sch * 128:(sch + 1) * 128],
                        wv_sb[:, dc, :],
                        start=(dc == 0), stop=(dc == 7))
                nc.scalar.copy(vs2[0:64, 2 * sch, :, :], ps_vs[0:64])
                nc.scalar.copy(vs2[64:128, 2 * sch + 1, :, :],
                               ps_vs[64:128])
                nc.sync.dma_start(vs2[64:128, 2 * sch, :, :],
                                  vs2[0:64, 2 * sch, :, :])
                nc.sync.dma_start(vs2[0:64, 2 * sch + 1, :, :],
                                  vs2[64:128, 2 * sch + 1, :, :])
            # block-diag summary-v stationaries for this vh's head pairs
            for hp in range(4 * vh, 4 * vh + 4):
                hl = 2 * hp - 8 * vh        # head index within vs2 cols
                nc.vector.tensor_copy(
                    vsd[0:64, hp * 4:hp * 4 + 4, 0:64],
                    vs2[0:64, :, hl, :])
                nc.vector.tensor_copy(
                    vsd[64:128, hp * 4:hp * 4 + 4, 64:128],
                    vs2[64:128, :, hl + 1, :])

        # ------------- P2 attention stages -------------
        def stage_scores(b, hp):
            c0 = b * 512
            s_loc = psq.tile([128, 1024], F32, tag="u1024", bufs=1)
            for hh in range(2):
                for p4 in range(4):
                    cq = c0 + p4 * 128
                    nc.tensor.matmul(
                        s_loc[:, hh * 512 + p4 * 128:
                              hh * 512 + (p4 + 1) * 128],
                        kT[hh * 64:hh * 64 + 64, hp, cq:cq + 128],
                        qT[hh * 64:hh * 64 + 64, hp, cq:cq + 128],
                        start=True, stop=True)
            s_sum = psq.tile([128, 512], F32, tag="u512", bufs=3)
            nc.tensor.matmul(s_sum[:], ksd[:, hp * 4 + b, :],
                             qT[:, hp, c0:c0 + 512],
                             start=True, stop=True)
            pml = p2.tile([128, 1024], BF16, tag="pml")
            nc.scalar.activation(pml[:], s_loc[:], AF.Exp, scale=SCALE)
            pms = p2.tile([128, 512], BF16, tag="pms")
            nc.scalar.activation(pms[:], s_sum[:], AF.Exp, scale=SCALE)
            nc.gpsimd.tensor_mul(pml[:], pml[:], mloc2_sb[:])
            nc.vector.tensor_mul(pms[:], pms[:], msum2_sb[:])
            return pml, pms

        def stage_av(b, hp, pml, pms):
            c0 = b * 512
            l_bc = psq.tile([128, 512], F32, tag="u512", bufs=3)
            nc.tensor.matmul(l_bc[:], cst_sb[:, 0:128], pms[:],
                             start=True, stop=False, skip_group_check=True)
            nc.tensor.matmul(l_bc[0:64, :], cst_sb[:, 128:192],
                             pml[:, 0:512],
                             start=False, stop=False, skip_group_check=True)
            nc.tensor.matmul(l_bc[64:128, :], cst_sb[:, 128:192],
                             pml[:, 512:1024],
                             start=False, stop=True, skip_group_check=True)
            rinv = p2.tile([128, 512], BF16, tag="rinv")
            with nc.allow_low_precision(reason="bf16 softmax recip"):
                nc.vector.reciprocal(rinv[:], l_bc[:])
            av = psq.tile([128, 512], F32, tag="u512", bufs=3)
            nc.tensor.matmul(av[:], vsd[:, hp * 4 + b, :], pms[:],
                             start=True, stop=False, skip_group_check=True)
            for hh in range(2):
                for p4 in range(4):
                    nc.tensor.matmul(
                        av[hh * 64:hh * 64 + 64, p4 * 128:(p4 + 1) * 128],
                        v_sb[:, b * 4 + p4, 2 * hp + hh, :],
                        pml[:, hh * 512 + p4 * 128:
                            hh * 512 + (p4 + 1) * 128],
                        start=False, stop=(p4 == 3),
                        skip_group_check=True)
            nc.vector.tensor_mul(attnT[:, hp, c0:c0 + 512], av[:], rinv[:])

        def stage_p3(b):
            c0 = b * 512
            for oc in range(8):
                ps_o = psq.tile([128, 512], F32, tag="p512", bufs=3)
                for hp in range(8):
                    nc.tensor.matmul(
                        ps_o[:],
                        woT_sb[:, hp, oc * 128:(oc + 1) * 128],
                        attnT[:, hp, c0:c0 + 512],
                        start=(hp == 0), stop=(hp == 7))
                o_sb = p2.tile([128, 512], BF16, tag="o_sb", bufs=2)
                nc.scalar.activation(o_sb[:], ps_o[:], AF.Identity,
                                     bias=boT_sb[:, oc:oc + 1])
                nc.sync.dma_start(
                    outT[oc * 128:(oc + 1) * 128, c0:c0 + 512], o_sb[:])

        def run_wave(pairs, with_p3):
            pend = stage_scores(*pairs[0])
            for i, (b, hp) in enumerate(pairs):
                if i + 1 < len(pairs):
                    nxt = stage_scores(*pairs[i + 1])
                else:
                    nxt = None
                stage_av(b, hp, *pend)
                pend = nxt
                if with_p3 and hp == 7:
                    stage_p3(b)

        do2 = 2 in phases
        do3 = 3 in phases
        if 1 in phases:
            p1_half(0)
            if do2:
                run_wave([(b, hp) for b in range(B) for hp in range(4)],
                         False)
            p1_half(1)
            px_cm.__exit__(None, None, None)
            pl = ctx.enter_context(tc_.tile_pool(name="late", bufs=1))
            woT_sb = pl.tile([128, 8, D], BF16, tag="woT")
            nc.sync.dma_start(woT_sb[:], woT[:])
            if do2:
                run_wave([(b, hp) for b in range(B) for hp in range(4, 8)],
                         do3)
            elif do3:
                for b in range(B):
                    stage_p3(b)

    nc.compile()
    return nc


def make_in_maps(x, in_proj_weight, in_proj_bias, out_proj_weight,
                 out_proj_bias):
    f32, bf16 = np.float32, ml_dtypes.bfloat16
    x = np.asarray(x, f32)
    fp8 = ml_dtypes.float8_e4m3
    wiT = np.asarray(in_proj_weight, f32).T              # [D, 3D]
    # [hp*128+p, dc, c] layouts so each weight loads in one DMA
    wq8 = np.ascontiguousarray(
        (wiT[:, :D] * 4096.0).astype(fp8).reshape(8, 128, 8, 128)
        .transpose(2, 1, 0, 3).reshape(D, 8, 128))
    wk8 = np.ascontiguousarray(
        (wiT[:, D:2 * D] * 4096.0).astype(fp8).reshape(8, 128, 8, 128)
        .transpose(2, 1, 0, 3).reshape(D, 8, 128))
    wkT = np.ascontiguousarray(
        wiT[:, D:2 * D].astype(bf16).reshape(8, 128, 8, 128)
        .transpose(2, 1, 0, 3).reshape(D, 8, 128))
    wvT = np.ascontiguousarray(
        wiT[:, 2 * D:].astype(bf16).reshape(8, 128, 2, 512)
        .transpose(2, 1, 0, 3).reshape(256, 8, 512))
    bi = np.asarray(in_proj_bias, f32)
    biT = np.ascontiguousarray(bi[:2 * D].reshape(16, 128).T)
    wo = np.asarray(out_proj_weight, f32)
    woT = np.ascontiguousarray(
        wo.T.astype(bf16).reshape(8, 128, D).transpose(1, 0, 2))
    bop = wo @ bi[2 * D:] + np.asarray(out_proj_bias, f32)
    boT = np.ascontiguousarray(bop.reshape(8, 128).T)

    p = np.arange(128)
    cstN = np.zeros((128, 192), f32)
    cstN[:, 0:128] = ((p[:, None] < 64) == (p[None, :] < 64))
    cstN[:, 128:192] = 1.0
    cstN = cstN.astype(bf16)

    k2 = np.arange(128)[:, None]
    q = np.arange(SC)[None, :]
    mloc = (((k2 // 64) == ((q // 64) % 2)) & ((q % 64) >= (k2 % 64)))
    mloc2 = np.tile(mloc.astype(f32), (1, 2)).astype(bf16)

    xs = x[:, BLK - 1::BLK, :]                           # [B, 64, D]
    xsTf = xs.transpose(2, 0, 1).reshape(D, NSUM)
    xsT = np.ascontiguousarray(
        xsTf.astype(bf16).reshape(8, 128, NSUM).transpose(1, 0, 2))
    xs8 = np.ascontiguousarray(
        (xsTf * 16.0).astype(fp8).reshape(8, 128, NSUM).transpose(1, 0, 2))

    m = np.arange(64)[:, None]
    in_maps = []
    for c in range(NCORES):
        xc = x[:, c * SC:(c + 1) * SC, :]                # [B, 512, D]
        xTf = xc.transpose(2, 0, 1).reshape(D, TC)
        xTc = np.ascontiguousarray(
            xTf.astype(bf16).reshape(8, 128, TC).transpose(1, 0, 2))
        xT8c = np.ascontiguousarray(
            (xTf * 16.0).astype(fp8).reshape(8, 128, TC).transpose(1, 0, 2))
        ms = (m < (c * BPC + (q // 64))).astype(f32)     # [64, 512]
        msum2 = np.concatenate([ms, ms], 0).astype(bf16)
        in_maps.append({
            "xT": xTc, "xT8": xT8c, "xsT": xsT, "xs8": xs8, "wq8": wq8,
            "wk8": wk8, "wkT": wkT, "wvT": wvT, "biT": biT,
            "woT": woT, "boT": boT, "cstN": cstN,
            "mloc2": mloc2, "msum2": msum2,
        })
    return in_maps


_NC_CACHE = []


def kernel(x, in_proj_weight, in_proj_bias, out_proj_weight, out_proj_bias):
    if not _NC_CACHE:
        _NC_CACHE.append(build_nc())
    nc = _NC_CACHE[0]
    in_maps = make_in_maps(x, in_proj_weight, in_proj_bias, out_proj_weight,
                           out_proj_bias)
    res = run_bass_kernel_spmd(nc, in_maps, core_ids=list(range(NCORES)))
    out = np.empty((B, S, D), np.float32)
    for c in range(NCORES):
        oT = np.asarray(res.results[c]["outT"]).astype(np.float32)
        out[:, c * SC:(c + 1) * SC, :] = \
            oT.reshape(D, B, SC).transpose(1, 2, 0)
    return out


# revision 23
# speedup vs baseline: 1.2362x; 1.0253x over previous
"""CronRoot (sqrt-N block-sparse causal) multihead attention on 8 trn2 cores.

v3 (from v2 base): k-projection moves to fp8 DoubleRow like q (measured
rel-err 1.84e-2 vs the 2e-2 gate, verified against a bit-matched numpy
emulation of the kernel numerics); softmax denominator is computed with
ones-block stationaries that broadcast l directly to [128,512] PSUM,
dropping the separate broadcast matmul, the [2,512] reciprocal and the
av PSUM->SBUF staging copy; the three phases share one pool scope so the
tile scheduler overlaps wave-1 attention with the second half of the
projections; DMA instruction count cut ~5x by host-side re-layout of
weights/x so each tensor loads with one large-descriptor DMA.

Sharding: sequence-parallel. Each core owns 8 of the 64 blocks (512
positions) for all batches/heads; summary k/v recomputed per-core from the
256 summary rows of x (no collectives).

Engine split per (b, head-pair) attention instance:
  PE: 8 local-score mm, 1 summary-score mm (block-diag ksd), 3 denominator
      mm (broadcast-l), 9 AV mm (block-diag vsd + 8 local).
  Scalar: exp(local [128,1024]), exp(summary [128,512]).
  GpSimd: local mask multiply. DVE: summary mask multiply, reciprocal
  [128,512], final (av*1/l) -> bf16 attnT.
"""

import numpy as np
import ml_dtypes
from contextlib import ExitStack

import concourse.bass as bass  # noqa: F401
import concourse.tile as tile
from concourse import bacc, mybir
from concourse.bass_utils import run_bass_kernel_spmd

F32 = mybir.dt.float32
FP8 = mybir.dt.float8e4
DR = mybir.MatmulPerfMode.DoubleRow
QSC = 1.0 / 65536.0  # undo x*16 and w*4096 scaling
FP16 = mybir.dt.bfloat16  # fp16 matmuls measured ~2x slower on HW
AF = mybir.ActivationFunctionType

B, S, D = 4, 4096, 1024
H, HD = 16, 64
BLK = 64                 # block size (= sqrt(S))
NB = S // BLK            # 64 blocks
NCORES = 8
SC = S // NCORES         # 512 seq positions per core
BPC = NB // NCORES       # 8 blocks per core
TC = B * SC              # 2048 (b-major) t columns per core
NSUM = B * NB            # 256 summary positions (b-major)
SCALE = 1.0 / np.sqrt(HD)


def build_nc(repeat=1, phases=(1, 2, 3), k_fp8=True):
    nc = bacc.Bacc("TRN2", target_bir_lowering=False, debug=False,
                   num_devices=NCORES)

    xT = nc.dram_tensor("xT", [128, 8, TC], FP16, kind="ExternalInput").ap()
    xT8 = nc.dram_tensor("xT8", [128, 8, TC], FP8, kind="ExternalInput").ap()
    xsT = nc.dram_tensor("xsT", [128, 8, NSUM], FP16,
                         kind="ExternalInput").ap()
    xs8 = nc.dram_tensor("xs8", [128, 8, NSUM], FP8,
                         kind="ExternalInput").ap()
    wq8 = nc.dram_tensor("wq8", [D, 8, 128], FP8, kind="ExternalInput").ap()
    wk8 = nc.dram_tensor("wk8", [D, 8, 128], FP8, kind="ExternalInput").ap()
    wkT = nc.dram_tensor("wkT", [D, 8, 128], FP16, kind="ExternalInput").ap()
    wvT = nc.dram_tensor("wvT", [256, 8, 512], FP16,
                         kind="ExternalInput").ap()
    biT = nc.dram_tensor("biT", [128, 16], F32, kind="ExternalInput").ap()
    woT = nc.dram_tensor("woT", [128, 8, D], FP16, kind="ExternalInput").ap()
    boT = nc.dram_tensor("boT", [128, 8], F32, kind="ExternalInput").ap()
    cstN = nc.dram_tensor("cstN", [128, 192], FP16, kind="ExternalInput").ap()
    mcomb = nc.dram_tensor("mcomb", [128, 1536], FP16,
                           kind="ExternalInput").ap()
    outT = nc.dram_tensor("outT", [D, TC], FP16, kind="ExternalOutput").ap()

    with tile.TileContext(nc) as tc_:
      for _rep in range(repeat):
       with ExitStack() as ctx:
        pp = ctx.enter_context(tc_.tile_pool(name="persist", bufs=1))
        qT = pp.tile([128, 8, TC], FP16, tag="qT")
        kT = pp.tile([128, 8, TC], FP16, tag="kT")
        v_sb = pp.tile([128, 16, 16, HD], FP16, tag="v")     # (tcn, h, d)
        ksd = pp.tile([128, 32, 128], FP16, tag="ksd")       # (hp*4+b) diag
        vsd = pp.tile([128, 32, 128], FP16, tag="vsd")
        attnT = pp.tile([128, 8, TC], FP16, tag="attnT")

        pw = ctx.enter_context(tc_.tile_pool(name="wstage", bufs=2))
        p2 = ctx.enter_context(tc_.tile_pool(name="p2", bufs=2))
        psq = ctx.enter_context(tc_.tile_pool(name="ps", bufs=1,
                                              space="PSUM"))
        px_cm = tc_.tile_pool(name="xstage", bufs=1)
        px = px_cm.__enter__()

        # critical-path loads first: x chunks feed the first q/k matmuls;
        # split across the SP and Activation HWDGE queues to halve latency
        xT8_sb = px.tile([128, 8, TC], FP8, tag="xT8")
        for dc in range(4):
            eng = nc.sync if dc % 2 == 0 else nc.scalar
            eng.dma_start(xT8_sb[:, 2 * dc:2 * dc + 2, :],
                          xT8[:, 2 * dc:2 * dc + 2, :])
        biT_sb = pp.tile([128, 16], F32, tag="biT")
        nc.sync.dma_start(biT_sb[:], biT[:])
        xsT_sb = px.tile([128, 8, NSUM], FP16, tag="xsT")
        nc.scalar.dma_start(xsT_sb[:], xsT[:])
        xs8_sb = px.tile([128, 8, NSUM], FP8, tag="xs8")
        nc.scalar.dma_start(xs8_sb[:], xs8[:])
        boT_sb = pp.tile([128, 8], F32, tag="boT")
        cst_sb = pp.tile([128, 192], FP16, tag="cstN")
        mcomb_sb = pp.tile([128, 1536], FP16, tag="mcomb")
        nc.vector.memset(ksd[:], 0.0)
        nc.vector.memset(vsd[:], 0.0)

        # ---------------- P1 projections for one half ----------------
        def p1_half(vh):
            for hp in range(4 * vh, 4 * vh + 4):
                # q chunk: fp8 DoubleRow (x*16, wq*4096)
                wq_sb = pw.tile([128, 8, 128], FP8, tag="w8_sb")
                nc.sync.dma_start(wq_sb[:],
                                  wq8[hp * 128:(hp + 1) * 128, :, :])
                for tt in range(4):
                    ps_qk = psq.tile([128, 512], F32, tag="p512", bufs=3)
                    for kp in range(4):
                        nc.tensor.matmul(
                            ps_qk[:],
                            wq_sb[:, 2 * kp:2 * kp + 2, :],
                            xT8_sb[:, 2 * kp:2 * kp + 2,
                                   tt * 512:(tt + 1) * 512],
                            start=(kp == 0), stop=(kp == 3),
                            perf_mode=DR)
                    nc.vector.tensor_scalar(
                        qT[:, hp, tt * 512:(tt + 1) * 512], ps_qk[:],
                        QSC, biT_sb[:, hp:hp + 1],
                        mybir.AluOpType.mult, mybir.AluOpType.add)
                # k chunk (+ summary keys into block-diag ksd)
                if k_fp8:
                    wk_sb = pw.tile([128, 8, 128], FP8, tag="wk_sb")
                    nc.sync.dma_start(wk_sb[:],
                                      wk8[hp * 128:(hp + 1) * 128, :, :])
                    for tt in range(4):
                        ps_qk = psq.tile([128, 512], F32, tag="p512", bufs=3)
                        for kp in range(4):
                            nc.tensor.matmul(
                                ps_qk[:],
                                wk_sb[:, 2 * kp:2 * kp + 2, :],
                                xT8_sb[:, 2 * kp:2 * kp + 2,
                                       tt * 512:(tt + 1) * 512],
                                start=(kp == 0), stop=(kp == 3),
                                perf_mode=DR)
                        nc.vector.tensor_scalar(
                            kT[:, hp, tt * 512:(tt + 1) * 512], ps_qk[:],
                            QSC, biT_sb[:, hp + 8:hp + 9],
                            mybir.AluOpType.mult, mybir.AluOpType.add)
                    ps_ks = psq.tile([128, 4, 64], F32, tag="p512", bufs=3)
                    for kp in range(4):
                        nc.tensor.matmul(
                            ps_ks[:],
                            wk_sb[:, 2 * kp:2 * kp + 2, :],
                            xs8_sb[:, 2 * kp:2 * kp + 2, :],
                            start=(kp == 0), stop=(kp == 3),
                            perf_mode=DR)
                    ksc = QSC
                else:
                    wk_sb = pw.tile([128, 8, 128], FP16, tag="wk_sb")
                    nc.sync.dma_start(wk_sb[:],
                                      wkT[hp * 128:(hp + 1) * 128, :, :])
                    for tt in range(4):
                        kx = pw.tile([128, 8, 512], FP16, tag="kx")
                        nc.sync.dma_start(kx[:],
                                          xT[:, :, tt * 512:(tt + 1) * 512])
                        ps_qk = psq.tile([128, 512], F32, tag="p512", bufs=3)
                        for dc in range(8):
                            nc.tensor.matmul(
                                ps_qk[:], wk_sb[:, dc, :],
                                kx[:, dc, :],
                                start=(dc == 0), stop=(dc == 7))
                        nc.scalar.activation(
                            kT[:, hp, tt * 512:(tt + 1) * 512], ps_qk[:],
                            AF.Identity, bias=biT_sb[:, hp + 8:hp + 9])
                    ps_ks = psq.tile([128, 4, 64], F32, tag="p512", bufs=3)
                    for dc in range(8):
                        nc.tensor.matmul(ps_ks[:], wk_sb[:, dc, :],
                                         xsT_sb[:, dc, :],
                                         start=(dc == 0), stop=(dc == 7))
                    ksc = 1.0
                nc.scalar.activation(
                    ksd[0:64, hp * 4:hp * 4 + 4, 0:64], ps_ks[0:64],
                    AF.Identity, scale=ksc, bias=biT_sb[0:64, hp + 8:hp + 9])
                nc.scalar.activation(
                    ksd[64:128, hp * 4:hp * 4 + 4, 64:128], ps_ks[64:128],
                    AF.Identity, scale=ksc,
                    bias=biT_sb[64:128, hp + 8:hp + 9])

            # v features for this half (heads 8*vh .. 8*vh+8)
            wv_sb = pw.tile([128, 8, 512], FP16, tag="wv_sb", bufs=1)
            nc.sync.dma_start(wv_sb[:], wvT[vh * 128:(vh + 1) * 128, :, :])
            for tcn in range(16):
                xch = pw.tile([128, 8, 128], FP16, tag="xch", bufs=3)
                nc.sync.dma_start(xch[:],
                                  xT[:, :, tcn * 128:(tcn + 1) * 128])
                ps_v = psq.tile([128, 512], F32, tag="p512", bufs=3)
                for dc in range(8):
                    nc.tensor.matmul(
                        ps_v[:],
                        xch[:, dc, :],
                        wv_sb[:, dc, :],
                        start=(dc == 0), stop=(dc == 7))
                nc.scalar.copy(
                    v_sb[:, tcn, vh * 8:(vh + 1) * 8, :], ps_v[:])
            # summary v -> vs2 (with duplicated partition halves)
            vs2 = px.tile([128, 4, 8, HD], FP16, tag="vs2")
            for sch in range(2):
                ps_vs = psq.tile([128, 512], F32, tag="p512", bufs=3)
                for dc in range(8):
                    nc.tensor.matmul(
                        ps_vs[:],
                        xsT_sb[:, dc, sch * 128:(sch + 1) * 128],
                        wv_sb[:, dc, :],
                        start=(dc == 0), stop=(dc == 7))
                nc.scalar.copy(vs2[0:64, 2 * sch, :, :], ps_vs[0:64])
                nc.scalar.copy(vs2[64:128, 2 * sch + 1, :, :],
                               ps_vs[64:128])
                nc.sync.dma_start(vs2[64:128, 2 * sch, :, :],
                                  vs2[0:64, 2 * sch, :, :])
                nc.sync.dma_start(vs2[0:64, 2 * sch + 1, :, :],
                                  vs2[64:128, 2 * sch + 1, :, :])
            # block-diag summary-v stationaries for this vh's head pairs
            for hp in range(4 * vh, 4 * vh + 4):
                hl = 2 * hp - 8 * vh        # head index within vs2 cols
                nc.vector.tensor_copy(
                    vsd[0:64, hp * 4:hp * 4 + 4, 0:64],
                    vs2[0:64, :, hl, :])
                nc.vector.tensor_copy(
                    vsd[64:128, hp * 4:hp * 4 + 4, 64:128],
                    vs2[64:128, :, hl + 1, :])

        # ------------- P2 attention stages -------------
        def stage_scores(b, hp):
            c0 = b * 512
            s_loc = psq.tile([128, 1024], F32, tag="u1024", bufs=1)
            for hh in range(2):
                for p4 in range(4):
                    cq = c0 + p4 * 128
                    nc.tensor.matmul(
                        s_loc[:, hh * 512 + p4 * 128:
                              hh * 512 + (p4 + 1) * 128],
                        kT[hh * 64:hh * 64 + 64, hp, cq:cq + 128],
                        qT[hh * 64:hh * 64 + 64, hp, cq:cq + 128],
                        start=True, stop=True)
            s_sum = psq.tile([128, 512], F32, tag="u512", bufs=3)
            nc.tensor.matmul(s_sum[:], ksd[:, hp * 4 + b, :],
                             qT[:, hp, c0:c0 + 512],
                             start=True, stop=True)
            pml = p2.tile([128, 1024], FP16, tag="pml")
            nc.scalar.activation(pml[:], s_loc[:], AF.Exp, scale=SCALE)
            pms = p2.tile([128, 512], FP16, tag="pms")
            nc.scalar.activation(pms[:], s_sum[:], AF.Exp, scale=SCALE)
            nc.gpsimd.tensor_mul(pml[:], pml[:], mcomb_sb[:, 0:1024])
            nc.vector.tensor_mul(pms[:], pms[:], mcomb_sb[:, 1024:1536])
            return pml, pms

        def stage_av(b, hp, pml, pms):
            c0 = b * 512
            l_bc = psq.tile([128, 512], F32, tag="u512", bufs=3)
            nc.tensor.matmul(l_bc[:], cst_sb[:, 0:128], pms[:],
                             start=True, stop=False, skip_group_check=True)
            nc.tensor.matmul(l_bc[0:64, :], cst_sb[:, 128:192],
                             pml[:, 0:512],
                             start=False, stop=False, skip_group_check=True)
            nc.tensor.matmul(l_bc[64:128, :], cst_sb[:, 128:192],
                             pml[:, 512:1024],
                             start=False, stop=True, skip_group_check=True)
            rinv = p2.tile([128, 512], FP16, tag="rinv")
            with nc.allow_low_precision(reason="fp16 softmax recip"):
                nc.vector.reciprocal(rinv[:], l_bc[:])
            av = psq.tile([128, 512], F32, tag="u512", bufs=3)
            nc.tensor.matmul(av[:], vsd[:, hp * 4 + b, :], pms[:],
                             start=True, stop=False, skip_group_check=True)
            for hh in range(2):
                for p4 in range(4):
                    nc.tensor.matmul(
                        av[hh * 64:hh * 64 + 64, p4 * 128:(p4 + 1) * 128],
                        v_sb[:, b * 4 + p4, 2 * hp + hh, :],
                        pml[:, hh * 512 + p4 * 128:
                            hh * 512 + (p4 + 1) * 128],
                        start=False, stop=(p4 == 3),
                        skip_group_check=True)
            nc.vector.tensor_mul(attnT[:, hp, c0:c0 + 512], av[:], rinv[:])

        def stage_p3(b):
            c0 = b * 512
            for oc in range(8):
                ps_o = psq.tile([128, 512], F32, tag="p512", bufs=3)
                for hp in range(8):
                    nc.tensor.matmul(
                        ps_o[:],
                        woT_sb[:, hp, oc * 128:(oc + 1) * 128],
                        attnT[:, hp, c0:c0 + 512],
                        start=(hp == 0), stop=(hp == 7))
                o_sb = p2.tile([128, 512], FP16, tag="o_sb", bufs=2)
                nc.vector.tensor_scalar(o_sb[:], ps_o[:],
                                        boT_sb[:, oc:oc + 1], None,
                                        mybir.AluOpType.add)
                nc.sync.dma_start(
                    outT[oc * 128:(oc + 1) * 128, c0:c0 + 512], o_sb[:])

        def run_wave(pairs, with_p3):
            pend = stage_scores(*pairs[0])
            for i, (b, hp) in enumerate(pairs):
                if i + 1 < len(pairs):
                    nxt = stage_scores(*pairs[i + 1])
                else:
                    nxt = None
                stage_av(b, hp, *pend)
                pend = nxt
                if with_p3 and hp == 7:
                    stage_p3(b)

        do2 = 2 in phases
        do3 = 3 in phases
        if 1 in phases:
            p1_half(0)
            nc.sync.dma_start(cst_sb[:], cstN[:])
            nc.sync.dma_start(mcomb_sb[:], mcomb[:])
            nc.sync.dma_start(boT_sb[:], boT[:])
            if do2:
                run_wave([(b, hp) for b in range(B) for hp in range(4)],
                         False)
            p1_half(1)
            px_cm.__exit__(None, None, None)
            pl = ctx.enter_context(tc_.tile_pool(name="late", bufs=1))
            woT_sb = pl.tile([128, 8, D], FP16, tag="woT")
            nc.sync.dma_start(woT_sb[:], woT[:])
            if do2:
                run_wave([(b, hp) for b in range(B) for hp in range(4, 8)],
                         do3)
            elif do3:
                for b in range(B):
                    stage_p3(b)

    nc.compile()
    return nc


def make_in_maps(x, in_proj_weight, in_proj_bias, out_proj_weight,
                 out_proj_bias):
    f32, bf16 = np.float32, ml_dtypes.bfloat16
    x = np.asarray(x, f32)
    fp8 = ml_dtypes.float8_e4m3
    wiT = np.asarray(in_proj_weight, f32).T              # [D, 3D]
    # [hp*128+p, dc, c] layouts so each weight loads in one DMA
    wq8 = np.ascontiguousarray(
        (wiT[:, :D] * 4096.0).astype(fp8).reshape(8, 128, 8, 128)
        .transpose(2, 1, 0, 3).reshape(D, 8, 128))
    wk8 = np.ascontiguousarray(
        (wiT[:, D:2 * D] * 4096.0).astype(fp8).reshape(8, 128, 8, 128)
        .transpose(2, 1, 0, 3).reshape(D, 8, 128))
    wkT = np.ascontiguousarray(
        wiT[:, D:2 * D].astype(bf16).reshape(8, 128, 8, 128)
        .transpose(2, 1, 0, 3).reshape(D, 8, 128))
    wvT = np.ascontiguousarray(
        wiT[:, 2 * D:].astype(bf16).reshape(8, 128, 2, 512)
        .transpose(2, 1, 0, 3).reshape(256, 8, 512))
    bi = np.asarray(in_proj_bias, f32)
    biT = np.ascontiguousarray(bi[:2 * D].reshape(16, 128).T)
    wo = np.asarray(out_proj_weight, f32)
    woT = np.ascontiguousarray(
        wo.T.astype(bf16).reshape(8, 128, D).transpose(1, 0, 2))
    bop = wo @ bi[2 * D:] + np.asarray(out_proj_bias, f32)
    boT = np.ascontiguousarray(bop.reshape(8, 128).T)

    p = np.arange(128)
    cstN = np.zeros((128, 192), f32)
    cstN[:, 0:128] = ((p[:, None] < 64) == (p[None, :] < 64))
    cstN[:, 128:192] = 1.0
    cstN = cstN.astype(bf16)

    k2 = np.arange(128)[:, None]
    q = np.arange(SC)[None, :]
    mloc = (((k2 // 64) == ((q // 64) % 2)) & ((q % 64) >= (k2 % 64)))
    mloc2 = np.tile(mloc.astype(f32), (1, 2))

    xs = x[:, BLK - 1::BLK, :]                           # [B, 64, D]
    xsTf = xs.transpose(2, 0, 1).reshape(D, NSUM)
    xsT = np.ascontiguousarray(
        xsTf.astype(bf16).reshape(8, 128, NSUM).transpose(1, 0, 2))
    xs8 = np.ascontiguousarray(
        (xsTf * 16.0).astype(fp8).reshape(8, 128, NSUM).transpose(1, 0, 2))

    m = np.arange(64)[:, None]
    in_maps = []
    for c in range(NCORES):
        xc = x[:, c * SC:(c + 1) * SC, :]                # [B, 512, D]
        xTf = xc.transpose(2, 0, 1).reshape(D, TC)
        xTc = np.ascontiguousarray(
            xTf.astype(bf16).reshape(8, 128, TC).transpose(1, 0, 2))
        xT8c = np.ascontiguousarray(
            (xTf * 16.0).astype(fp8).reshape(8, 128, TC).transpose(1, 0, 2))
        ms = (m < (c * BPC + (q // 64))).astype(f32)     # [64, 512]
        mcomb = np.concatenate(
            [mloc2, np.concatenate([ms, ms], 0)], 1).astype(bf16)
        in_maps.append({
            "xT": xTc, "xT8": xT8c, "xsT": xsT, "xs8": xs8, "wq8": wq8,
            "wk8": wk8, "wkT": wkT, "wvT": wvT, "biT": biT,
            "woT": woT, "boT": boT, "cstN": cstN, "mcomb": mcomb,
        })
    return in_maps


_NC_CACHE = []


def kernel(x, in_proj_weight, in_proj_bias, out_proj_weight, out_proj_bias):
    if not _NC_CACHE:
        _NC_CACHE.append(build_nc())
    nc = _NC_CACHE[0]
    in_maps = make_in_maps(x, in_proj_weight, in_proj_bias, out_proj_weight,
                           out_proj_bias)
    res = run_bass_kernel_spmd(nc, in_maps, core_ids=list(range(NCORES)))
    out = np.empty((B, S, D), np.float32)
    for c in range(NCORES):
        oT = np.asarray(res.results[c]["outT"]).astype(np.float32)
        out[:, c * SC:(c + 1) * SC, :] = \
            oT.reshape(D, B, SC).transpose(1, 2, 0)
    return out


# revision 24
# speedup vs baseline: 2.5508x; 2.0634x over previous
"""CronRoot (sqrt-N block-sparse causal) multihead attention on 8 trn2 cores.

v5 (from v2 base):
- q AND k projections in fp8 DoubleRow (x*16, w*4096 scaling; measured HW
  rel-err 1.821e-2 vs the 2e-2 gate, tracked by a numpy emulation of the
  kernel numerics that matches HW to ~0.1%).
- softmax denominator broadcast directly to [128,512] PSUM via ones-block
  stationaries (no separate bcast matmul / staging copy); DVE reciprocal.
- one pool scope across phases: wave-1 attention (hp 0-3) overlaps the
  second projection half; P3(b) interleaves into wave 2.
- host-side re-layout so each weight/x tensor loads in one large-descriptor
  DMA, split across the SP + Activation HWDGE queues.
- q/k PSUM evacuations on DVE tensor_scalar (scale+bias) to unload the
  scalar engine (note: GPSIMD cannot read PSUM).
- bf16 for all 16-bit operands (fp16 matmuls measured ~2x slower on HW).

Sharding: sequence-parallel. Each core owns 8 of the 64 blocks (512
positions) for all batches/heads; summary k/v recomputed per-core from the
256 summary rows of x (no collectives).

Engine split per (b, head-pair) attention instance:
  PE: 8 local-score mm, 1 summary-score mm (block-diag ksd), 3 denominator
      mm (broadcast-l), 9 AV mm (block-diag vsd + 8 local).
  Scalar: exp(local [128,1024]), exp(summary [128,512]).
  GpSimd: local mask multiply. DVE: summary mask multiply, reciprocal
  [128,512], final (av*1/l) -> bf16 attnT.
"""

import numpy as np
import ml_dtypes
from contextlib import ExitStack

import concourse.bass as bass  # noqa: F401
import concourse.tile as tile
from concourse import bacc, mybir
from concourse.bass_utils import run_bass_kernel_spmd

F32 = mybir.dt.float32
FP8 = mybir.dt.float8e4
DR = mybir.MatmulPerfMode.DoubleRow
QSC = 1.0 / 65536.0  # undo x*16 and w*4096 scaling
FP16 = mybir.dt.bfloat16  # fp16 matmuls measured ~2x slower on HW
AF = mybir.ActivationFunctionType

B, S, D = 4, 4096, 1024
H, HD = 16, 64
BLK = 64                 # block size (= sqrt(S))
NB = S // BLK            # 64 blocks
NCORES = 8
SC = S // NCORES         # 512 seq positions per core
BPC = NB // NCORES       # 8 blocks per core
TC = B * SC              # 2048 (b-major) t columns per core
NSUM = B * NB            # 256 summary positions (b-major)
SCALE = 1.0 / np.sqrt(HD)


def build_nc(repeat=1, phases=(1, 2, 3), k_fp8=True):
    nc = bacc.Bacc("TRN2", target_bir_lowering=False, debug=False,
                   num_devices=NCORES)

    xT = nc.dram_tensor("xT", [128, 8, TC], FP16, kind="ExternalInput").ap()
    xT8 = nc.dram_tensor("xT8", [128, 8, TC], FP8, kind="ExternalInput").ap()
    xsT = nc.dram_tensor("xsT", [128, 8, NSUM], FP16,
                         kind="ExternalInput").ap()
    xs8 = nc.dram_tensor("xs8", [128, 8, NSUM], FP8,
                         kind="ExternalInput").ap()
    wq8 = nc.dram_tensor("wq8", [D, 8, 128], FP8, kind="ExternalInput").ap()
    wk8 = nc.dram_tensor("wk8", [D, 8, 128], FP8, kind="ExternalInput").ap()
    wkT = nc.dram_tensor("wkT", [D, 8, 128], FP16, kind="ExternalInput").ap()
    wvT = nc.dram_tensor("wvT", [256, 8, 512], FP16,
                         kind="ExternalInput").ap()
    biT = nc.dram_tensor("biT", [128, 16], F32, kind="ExternalInput").ap()
    woT = nc.dram_tensor("woT", [128, 8, D], FP16, kind="ExternalInput").ap()
    boT = nc.dram_tensor("boT", [128, 8], F32, kind="ExternalInput").ap()
    cstN = nc.dram_tensor("cstN", [128, 192], FP16, kind="ExternalInput").ap()
    mcomb = nc.dram_tensor("mcomb", [128, 1536], FP16,
                           kind="ExternalInput").ap()
    outT = nc.dram_tensor("outT", [D, TC], FP16, kind="ExternalOutput").ap()

    with tile.TileContext(nc) as tc_:
      for _rep in range(repeat):
       with ExitStack() as ctx:
        pp = ctx.enter_context(tc_.tile_pool(name="persist", bufs=1))
        qT = pp.tile([128, 8, TC], FP16, tag="qT")
        kT = pp.tile([128, 8, TC], FP16, tag="kT")
        v_sb = pp.tile([128, 16, 16, HD], FP16, tag="v")     # (tcn, h, d)
        ksd = pp.tile([128, 32, 128], FP16, tag="ksd")       # (hp*4+b) diag
        vsd = pp.tile([128, 32, 128], FP16, tag="vsd")
        attnT = pp.tile([128, 8, TC], FP16, tag="attnT")

        pw = ctx.enter_context(tc_.tile_pool(name="wstage", bufs=2))
        p2 = ctx.enter_context(tc_.tile_pool(name="p2", bufs=2))
        psq = ctx.enter_context(tc_.tile_pool(name="ps", bufs=1,
                                              space="PSUM"))
        px_cm = tc_.tile_pool(name="xstage", bufs=1)
        px = px_cm.__enter__()

        # critical-path loads first: x chunks feed the first q/k matmuls;
        # split across the SP and Activation HWDGE queues to halve latency
        xT8_sb = px.tile([128, 8, TC], FP8, tag="xT8")
        for dc in range(4):
            eng = nc.sync if dc % 2 == 0 else nc.scalar
            eng.dma_start(xT8_sb[:, 2 * dc:2 * dc + 2, :],
                          xT8[:, 2 * dc:2 * dc + 2, :])
        biT_sb = pp.tile([128, 16], F32, tag="biT")
        nc.sync.dma_start(biT_sb[:], biT[:])
        xsT_sb = px.tile([128, 8, NSUM], FP16, tag="xsT")
        nc.scalar.dma_start(xsT_sb[:], xsT[:])
        xs8_sb = px.tile([128, 8, NSUM], FP8, tag="xs8")
        nc.scalar.dma_start(xs8_sb[:], xs8[:])
        boT_sb = pp.tile([128, 8], F32, tag="boT")
        cst_sb = pp.tile([128, 192], FP16, tag="cstN")
        mcomb_sb = pp.tile([128, 1536], FP16, tag="mcomb")
        nc.vector.memset(ksd[:], 0.0)
        nc.vector.memset(vsd[:], 0.0)

        # ---------------- P1 projections for one half ----------------
        def p1_half(vh):
            for hp in range(4 * vh, 4 * vh + 4):
                # q chunk: fp8 DoubleRow (x*16, wq*4096)
                wq_sb = pw.tile([128, 8, 128], FP8, tag="w8_sb")
                nc.sync.dma_start(wq_sb[:],
                                  wq8[hp * 128:(hp + 1) * 128, :, :])
                for tt in range(4):
                    ps_qk = psq.tile([128, 512], F32, tag="p512", bufs=3)
                    for kp in range(4):
                        nc.tensor.matmul(
                            ps_qk[:],
                            wq_sb[:, 2 * kp:2 * kp + 2, :],
                            xT8_sb[:, 2 * kp:2 * kp + 2,
                                   tt * 512:(tt + 1) * 512],
                            start=(kp == 0), stop=(kp == 3),
                            perf_mode=DR)
                    nc.vector.tensor_scalar(
                        qT[:, hp, tt * 512:(tt + 1) * 512], ps_qk[:],
                        QSC, biT_sb[:, hp:hp + 1],
                        mybir.AluOpType.mult, mybir.AluOpType.add)
                # k chunk (+ summary keys into block-diag ksd)
                if k_fp8:
                    wk_sb = pw.tile([128, 8, 128], FP8, tag="wk_sb")
                    nc.sync.dma_start(wk_sb[:],
                                      wk8[hp * 128:(hp + 1) * 128, :, :])
                    for tt in range(4):
                        ps_qk = psq.tile([128, 512], F32, tag="p512", bufs=3)
                        for kp in range(4):
                            nc.tensor.matmul(
                                ps_qk[:],
                                wk_sb[:, 2 * kp:2 * kp + 2, :],
                                xT8_sb[:, 2 * kp:2 * kp + 2,
                                       tt * 512:(tt + 1) * 512],
                                start=(kp == 0), stop=(kp == 3),
                                perf_mode=DR)
                        nc.vector.tensor_scalar(
                            kT[:, hp, tt * 512:(tt + 1) * 512], ps_qk[:],
                            QSC, biT_sb[:, hp + 8:hp + 9],
                            mybir.AluOpType.mult, mybir.AluOpType.add)
                    ps_ks = psq.tile([128, 4, 64], F32, tag="p512", bufs=3)
                    for kp in range(4):
                        nc.tensor.matmul(
                            ps_ks[:],
                            wk_sb[:, 2 * kp:2 * kp + 2, :],
                            xs8_sb[:, 2 * kp:2 * kp + 2, :],
                            start=(kp == 0), stop=(kp == 3),
                            perf_mode=DR)
                    ksc = QSC
                else:
                    wk_sb = pw.tile([128, 8, 128], FP16, tag="wk_sb")
                    nc.sync.dma_start(wk_sb[:],
                                      wkT[hp * 128:(hp + 1) * 128, :, :])
                    for tt in range(4):
                        kx = pw.tile([128, 8, 512], FP16, tag="kx")
                        nc.sync.dma_start(kx[:],
                                          xT[:, :, tt * 512:(tt + 1) * 512])
                        ps_qk = psq.tile([128, 512], F32, tag="p512", bufs=3)
                        for dc in range(8):
                            nc.tensor.matmul(
                                ps_qk[:], wk_sb[:, dc, :],
                                kx[:, dc, :],
                                start=(dc == 0), stop=(dc == 7))
                        nc.scalar.activation(
                            kT[:, hp, tt * 512:(tt + 1) * 512], ps_qk[:],
                            AF.Identity, bias=biT_sb[:, hp + 8:hp + 9])
                    ps_ks = psq.tile([128, 4, 64], F32, tag="p512", bufs=3)
                    for dc in range(8):
                        nc.tensor.matmul(ps_ks[:], wk_sb[:, dc, :],
                                         xsT_sb[:, dc, :],
                                         start=(dc == 0), stop=(dc == 7))
                    ksc = 1.0
                nc.scalar.activation(
                    ksd[0:64, hp * 4:hp * 4 + 4, 0:64], ps_ks[0:64],
                    AF.Identity, scale=ksc, bias=biT_sb[0:64, hp + 8:hp + 9])
                nc.scalar.activation(
                    ksd[64:128, hp * 4:hp * 4 + 4, 64:128], ps_ks[64:128],
                    AF.Identity, scale=ksc,
                    bias=biT_sb[64:128, hp + 8:hp + 9])

            # v features for this half (heads 8*vh .. 8*vh+8)
            wv_sb = pw.tile([128, 8, 512], FP16, tag="wv_sb", bufs=1)
            nc.sync.dma_start(wv_sb[:], wvT[vh * 128:(vh + 1) * 128, :, :])
            for tcn in range(16):
                xch = pw.tile([128, 8, 128], FP16, tag="xch", bufs=3)
                nc.sync.dma_start(xch[:],
                                  xT[:, :, tcn * 128:(tcn + 1) * 128])
                ps_v = psq.tile([128, 512], F32, tag="p512", bufs=3)
                for dc in range(8):
                    nc.tensor.matmul(
                        ps_v[:],
                        xch[:, dc, :],
                        wv_sb[:, dc, :],
                        start=(dc == 0), stop=(dc == 7))
                nc.scalar.copy(
                    v_sb[:, tcn, vh * 8:(vh + 1) * 8, :], ps_v[:])
            # summary v -> vs2 (with duplicated partition halves)
            vs2 = px.tile([128, 4, 8, HD], FP16, tag="vs2")
            for sch in range(2):
                ps_vs = psq.tile([128, 512], F32, tag="p512", bufs=3)
                for dc in range(8):
                    nc.tensor.matmul(
                        ps_vs[:],
                        xsT_sb[:, dc, sch * 128:(sch + 1) * 128],
                        wv_sb[:, dc, :],
                        start=(dc == 0), stop=(dc == 7))
                nc.scalar.copy(vs2[0:64, 2 * sch, :, :], ps_vs[0:64])
                nc.scalar.copy(vs2[64:128, 2 * sch + 1, :, :],
                               ps_vs[64:128])
                nc.sync.dma_start(vs2[64:128, 2 * sch, :, :],
                                  vs2[0:64, 2 * sch, :, :])
                nc.sync.dma_start(vs2[0:64, 2 * sch + 1, :, :],
                                  vs2[64:128, 2 * sch + 1, :, :])
            # block-diag summary-v stationaries for this vh's head pairs
            for hp in range(4 * vh, 4 * vh + 4):
                hl = 2 * hp - 8 * vh        # head index within vs2 cols
                nc.vector.tensor_copy(
                    vsd[0:64, hp * 4:hp * 4 + 4, 0:64],
                    vs2[0:64, :, hl, :])
                nc.vector.tensor_copy(
                    vsd[64:128, hp * 4:hp * 4 + 4, 64:128],
                    vs2[64:128, :, hl + 1, :])

        # ------------- P2 attention stages -------------
        def stage_scores(b, hp):
            c0 = b * 512
            s_loc = psq.tile([128, 1024], F32, tag="u1024", bufs=1)
            for hh in range(2):
                for p4 in range(4):
                    cq = c0 + p4 * 128
                    nc.tensor.matmul(
                        s_loc[:, hh * 512 + p4 * 128:
                              hh * 512 + (p4 + 1) * 128],
                        kT[hh * 64:hh * 64 + 64, hp, cq:cq + 128],
                        qT[hh * 64:hh * 64 + 64, hp, cq:cq + 128],
                        start=True, stop=True)
            s_sum = psq.tile([128, 512], F32, tag="u512", bufs=3)
            nc.tensor.matmul(s_sum[:], ksd[:, hp * 4 + b, :],
                             qT[:, hp, c0:c0 + 512],
                             start=True, stop=True)
            pml = p2.tile([128, 1024], FP16, tag="pml")
            nc.scalar.activation(pml[:], s_loc[:], AF.Exp, scale=SCALE)
            pms = p2.tile([128, 512], FP16, tag="pms")
            nc.scalar.activation(pms[:], s_sum[:], AF.Exp, scale=SCALE)
            nc.gpsimd.tensor_mul(pml[:], pml[:], mcomb_sb[:, 0:1024])
            nc.vector.tensor_mul(pms[:], pms[:], mcomb_sb[:, 1024:1536])
            return pml, pms

        def stage_av(b, hp, pml, pms):
            c0 = b * 512
            l_bc = psq.tile([128, 512], F32, tag="u512", bufs=3)
            nc.tensor.matmul(l_bc[:], cst_sb[:, 0:128], pms[:],
                             start=True, stop=False, skip_group_check=True)
            nc.tensor.matmul(l_bc[0:64, :], cst_sb[:, 128:192],
                             pml[:, 0:512],
                             start=False, stop=False, skip_group_check=True)
            nc.tensor.matmul(l_bc[64:128, :], cst_sb[:, 128:192],
                             pml[:, 512:1024],
                             start=False, stop=True, skip_group_check=True)
            rinv = p2.tile([128, 512], FP16, tag="rinv")
            with nc.allow_low_precision(reason="fp16 softmax recip"):
                nc.vector.reciprocal(rinv[:], l_bc[:])
            av = psq.tile([128, 512], F32, tag="u512", bufs=3)
            nc.tensor.matmul(av[:], vsd[:, hp * 4 + b, :], pms[:],
                             start=True, stop=False, skip_group_check=True)
            for hh in range(2):
                for p4 in range(4):
                    nc.tensor.matmul(
                        av[hh * 64:hh * 64 + 64, p4 * 128:(p4 + 1) * 128],
                        v_sb[:, b * 4 + p4, 2 * hp + hh, :],
                        pml[:, hh * 512 + p4 * 128:
                            hh * 512 + (p4 + 1) * 128],
                        start=False, stop=(p4 == 3),
                        skip_group_check=True)
            nc.vector.tensor_mul(attnT[:, hp, c0:c0 + 512], av[:], rinv[:])

        def stage_p3(b):
            c0 = b * 512
            for oc in range(8):
                ps_o = psq.tile([128, 512], F32, tag="p512", bufs=3)
                for hp in range(8):
                    nc.tensor.matmul(
                        ps_o[:],
                        woT_sb[:, hp, oc * 128:(oc + 1) * 128],
                        attnT[:, hp, c0:c0 + 512],
                        start=(hp == 0), stop=(hp == 7))
                o_sb = p2.tile([128, 512], FP16, tag="o_sb", bufs=2)
                nc.vector.tensor_scalar(o_sb[:], ps_o[:],
                                        boT_sb[:, oc:oc + 1], None,
                                        mybir.AluOpType.add)
                nc.sync.dma_start(
                    outT[oc * 128:(oc + 1) * 128, c0:c0 + 512], o_sb[:])

        def run_wave(pairs, with_p3):
            pend = stage_scores(*pairs[0])
            for i, (b, hp) in enumerate(pairs):
                if i + 1 < len(pairs):
                    nxt = stage_scores(*pairs[i + 1])
                else:
                    nxt = None
                stage_av(b, hp, *pend)
                pend = nxt
                if with_p3 and hp == 7:
                    stage_p3(b)

        do2 = 2 in phases
        do3 = 3 in phases
        if 1 in phases:
            p1_half(0)
            nc.sync.dma_start(cst_sb[:], cstN[:])
            nc.sync.dma_start(mcomb_sb[:], mcomb[:])
            nc.sync.dma_start(boT_sb[:], boT[:])
            if do2:
                run_wave([(b, hp) for b in range(B) for hp in range(4)],
                         False)
            p1_half(1)
            px_cm.__exit__(None, None, None)
            pl = ctx.enter_context(tc_.tile_pool(name="late", bufs=1))
            woT_sb = pl.tile([128, 8, D], FP16, tag="woT")
            nc.sync.dma_start(woT_sb[:], woT[:])
            if do2:
                run_wave([(b, hp) for b in range(B) for hp in range(4, 8)],
                         do3)
            elif do3:
                for b in range(B):
                    stage_p3(b)

    nc.compile()
    return nc


def make_in_maps(x, in_proj_weight, in_proj_bias, out_proj_weight,
                 out_proj_bias):
    f32, bf16 = np.float32, ml_dtypes.bfloat16
    x = np.asarray(x, f32)
    fp8 = ml_dtypes.float8_e4m3
    wiT = np.asarray(in_proj_weight, f32).T              # [D, 3D]
    # [hp*128+p, dc, c] layouts so each weight loads in one DMA
    wq8 = np.ascontiguousarray(
        (wiT[:, :D] * 4096.0).astype(fp8).reshape(8, 128, 8, 128)
        .transpose(2, 1, 0, 3).reshape(D, 8, 128))
    wk8 = np.ascontiguousarray(
        (wiT[:, D:2 * D] * 4096.0).astype(fp8).reshape(8, 128, 8, 128)
        .transpose(2, 1, 0, 3).reshape(D, 8, 128))
    wkT = np.ascontiguousarray(
        wiT[:, D:2 * D].astype(bf16).reshape(8, 128, 8, 128)
        .transpose(2, 1, 0, 3).reshape(D, 8, 128))
    wvT = np.ascontiguousarray(
        wiT[:, 2 * D:].astype(bf16).reshape(8, 128, 2, 512)
        .transpose(2, 1, 0, 3).reshape(256, 8, 512))
    bi = np.asarray(in_proj_bias, f32)
    biT = np.ascontiguousarray(bi[:2 * D].reshape(16, 128).T)
    wo = np.asarray(out_proj_weight, f32)
    woT = np.ascontiguousarray(
        wo.T.astype(bf16).reshape(8, 128, D).transpose(1, 0, 2))
    bop = wo @ bi[2 * D:] + np.asarray(out_proj_bias, f32)
    boT = np.ascontiguousarray(bop.reshape(8, 128).T)

    p = np.arange(128)
    cstN = np.zeros((128, 192), f32)
    cstN[:, 0:128] = ((p[:, None] < 64) == (p[None, :] < 64))
    cstN[:, 128:192] = 1.0
    cstN = cstN.astype(bf16)

    k2 = np.arange(128)[:, None]
    q = np.arange(SC)[None, :]
    mloc = (((k2 // 64) == ((q // 64) % 2)) & ((q % 64) >= (k2 % 64)))
    mloc2 = np.tile(mloc.astype(f32), (1, 2))

    xs = x[:, BLK - 1::BLK, :]                           # [B, 64, D]
    xsTf = xs.transpose(2, 0, 1).reshape(D, NSUM)
    xsT = np.ascontiguousarray(
        xsTf.astype(bf16).reshape(8, 128, NSUM).transpose(1, 0, 2))
    xs8 = np.ascontiguousarray(
        (xsTf * 16.0).astype(fp8).reshape(8, 128, NSUM).transpose(1, 0, 2))

    m = np.arange(64)[:, None]
    in_maps = []
    for c in range(NCORES):
        xc = x[:, c * SC:(c + 1) * SC, :]                # [B, 512, D]
        xTf = xc.transpose(2, 0, 1).reshape(D, TC)
        xTc = np.ascontiguousarray(
            xTf.astype(bf16).reshape(8, 128, TC).transpose(1, 0, 2))
        xT8c = np.ascontiguousarray(
            (xTf * 16.0).astype(fp8).reshape(8, 128, TC).transpose(1, 0, 2))
        ms = (m < (c * BPC + (q // 64))).astype(f32)     # [64, 512]
        mcomb = np.concatenate(
            [mloc2, np.concatenate([ms, ms], 0)], 1).astype(bf16)
        in_maps.append({
            "xT": xTc, "xT8": xT8c, "xsT": xsT, "xs8": xs8, "wq8": wq8,
            "wk8": wk8, "wkT": wkT, "wvT": wvT, "biT": biT,
            "woT": woT, "boT": boT, "cstN": cstN, "mcomb": mcomb,
        })
    return in_maps


_NC_CACHE = []


def kernel(x, in_proj_weight, in_proj_bias, out_proj_weight, out_proj_bias):
    if not _NC_CACHE:
        _NC_CACHE.append(build_nc())
    nc = _NC_CACHE[0]
    in_maps = make_in_maps(x, in_proj_weight, in_proj_bias, out_proj_weight,
                           out_proj_bias)
    res = run_bass_kernel_spmd(nc, in_maps, core_ids=list(range(NCORES)))
    out = np.empty((B, S, D), np.float32)
    for c in range(NCORES):
        oT = np.asarray(res.results[c]["outT"]).astype(np.float32)
        out[:, c * SC:(c + 1) * SC, :] = \
            oT.reshape(D, B, SC).transpose(1, 2, 0)
    return out
